# revision 71
# baseline (speedup 1.0000x reference)
"""Trainium2 Bass kernel for nn_Attention_43190191129190.

Model (per batch element b of 8):
    y   = x + dwconv3x3(x) + conv_b          (depthwise residual positional conv)
    qkv = y @ qkv_w.T ; split into q, k, v   (8 heads, dim 32)
    out = softmax(q k^T / sqrt(32)) v
    out = out @ out_w.T + out_b
Sharding: pure data-parallel, one batch element per NeuronCore (8 cores).

Per-core design (v5 — merged 2-bank tiles, wide exp, merged evacuations):

  The ACT+DVE engines are the wall: every S element must be exp'd
  (65536 cols of [128]-partition work) and every PSUM result must be
  evacuated by ACT/DVE (DMA and GPSIMD cannot touch PSUM). v5 cuts the
  per-instruction init overhead (ACT ~185ns, DVE ~125ns busy per op) by
  merging work into the widest possible instructions:

  1. x arrives bf16, host pre-transposed; 2 DMA-xbar transposes stage
     x^T, copied into a zero-haloed [C, 34, 34] image.
  2. diag conv matrices built on device by TWO [128,9,128]
     affine_selects (one per channel tile) instead of 18 narrow ones.
  3. conv per ct: one [128,2,512] PSUM tile, j halves as two 10-matmul
     accumulation groups, ONE [128,1024] evacuation (bias via K=1 tap).
  4. q^T/k^T per feature tile: one [128,2,512] tile, 4 matmuls, ONE
     evacuation. v: two 4-token-chunk units, 8 matmuls + ONE strided
     evacuation each into [v_h|1] 33-wide head slots (ones preset).
  5. Attention, head pair per generation, 8 m-steps each:
       S^T per (head, m): one [128,2,512] f32 PSUM tile (two 512-wide
       matmuls), then ONE 1024-wide exp:
         hs0 -> ACT exact Exp -> bf16; hs1 -> DVE Schraudolph
         (tensor_scalar s*A+B -> int16 bits == bf16(exp(s*SCALE))).
       PV unchanged: per-head [128, 8x33] PSUM accumulator, stationary
       p^T chunks, moving [v_h|1]; column 32 = softmax denominators;
       one accumulation group per bank. Norm per head: reciprocal +
       one broadcast tensor_tensor -> a_sb bf16.
  6. a_sb -> attnT: 8 transposes per ct share ONE [128,1024] bf16 bank
     (single accumulation group, disjoint regions), ONE 2x-mode DVE
     copy per ct.
  7. projection: chunk-1 + out_b staged mid-kernel into partial1 (pairs
     of token chunks share a bank, one [128,512] copy); tail re-adds
     partial1 via f32r identity matmuls and stores via merged copies.

  PSUM: 3x[128,2,512] f32 rotating slots + 2x[128,264] PV accumulators.
  Pre-attention work interleaves into the pair loops one self-contained
  slice per m-step (alloc+use+evacuate within the slice).
"""

import os

import numpy as np

import concourse.bass as bass
import concourse.tile as tile
from concourse import bacc, mybir
from concourse.bass_utils import run_bass_kernel_spmd

F32 = mybir.dt.float32
F32R = mybir.dt.float32r
BF16 = mybir.dt.bfloat16
I16 = mybir.dt.int16
AF = mybir.ActivationFunctionType
ALU = mybir.AluOpType

B, N, C = 8, 1024, 256
HEADS, DH = 8, 32
SCALE = DH ** -0.5
PAD = 34  # 32x32 spatial grid with 1-px halo

# blobA (bf16): id [128, 0:128] | w18 [128, 128:146] | convb cols [128, 146:148]
BA_ID, BA_W18, BA_CONVB, BAW = 0, 128, 146, 148
# blobB (bf16): outwT [128, 0:512] | outb row0 [512:768]
BB_OWT, BB_OUTB, BBW = 0, 512, 768

TAPS = [(ky, kx) for ky in range(3) for kx in range(3)]
# chunk-1 head pairs first so the chunk-1 projection can run mid-kernel;
# the tail then only waits on the last pair's (chunk-0) normalization
PAIRS = [(5, 7), (4, 6), (1, 3), (0, 2)]

# Schraudolph fast-exp: int16 bits of bf16(exp(s*SCALE)) = s*A + B
SCHR_C = 450000.0
SCHR_A = float(SCALE * (2 ** 23) / np.log(2) / 65536.0)
SCHR_B = float((127 * 2 ** 23 - SCHR_C) / 65536.0)


def build_nc(debug_dump=False):
    nc = bacc.Bacc("TRN2", target_bir_lowering=False, debug=False, num_devices=8)

    # x arrives host-pre-transposed: [C, N] bf16, one straight DMA
    x_d = nc.dram_tensor("x", (C, N), BF16, kind="ExternalInput").ap()
    qkvwT_d = nc.dram_tensor("qkv_wT", (C, 3 * C), F32R, kind="ExternalInput").ap()
    blobA_d = nc.dram_tensor("blobA", (128, BAW), BF16, kind="ExternalInput").ap()
    blobB_d = nc.dram_tensor("blobB", (128, BBW), BF16, kind="ExternalInput").ap()
    out_d = nc.dram_tensor("out", (N, C), F32, kind="ExternalOutput").ap()
    dbg = {}
    if debug_dump:
        for name, shape in (
            ("d_yT", (128, 2, N)), ("d_qT", (128, 2, N)), ("d_kT", (128, 2, N)),
            ("d_v", (128, 8, 264)), ("d_asb", (128, 8, 256)),
        ):
            dbg[name] = nc.dram_tensor(name, shape, F32, kind="ExternalOutput").ap()

    with tile.TileContext(nc) as tc:
        with (
            tc.tile_pool(name="const", bufs=1) as const,
            tc.tile_pool(name="big", bufs=1) as big,
            tc.tile_pool(name="pT", bufs=16) as ppool,
            tc.tile_pool(name="rcp", bufs=4) as rcp_p,
            tc.tile_pool(name="outs", bufs=4) as outs_p,
            tc.tile_pool(name="pst", bufs=3, space="PSUM") as pst,
            tc.tile_pool(name="pap", bufs=2, space="PSUM") as pap,
        ):
            # ---- persistent activations (x image first: DMA critical path)
            xpadT = big.tile([128, 2, PAD * PAD], BF16, tag="xpadT")
            xpv = xpadT.bitcast(mybir.dt.uint16).rearrange(
                "p ct (h w) -> p ct h w", h=PAD
            )
            nc.vector.memset(xpv[:, :, 0, :], 0)
            nc.vector.memset(xpv[:, :, PAD - 1, :], 0)
            nc.vector.memset(xpv[:, :, :, 0], 0)
            nc.vector.memset(xpv[:, :, :, PAD - 1], 0)

            # ---- DMAs. Per-DMA cost in the serial DMA pipeline is large
            # (HWDGE 625 + DGE delay 650 + transfer + completion sem 900),
            # so x is host-pre-transposed and lands in ONE straight DMA.
            blobA_sb = const.tile([128, BAW], BF16, tag="blobA")
            nc.sync.dma_start(blobA_sb, blobA_d)
            xstg = big.tile([128, 2, N], BF16, tag="xstg")
            for ct in range(2):
                nc.sync.dma_start(xstg[:, ct, :],
                                  x_d[ct * 128:(ct + 1) * 128, :])
            id_sb = blobA_sb[:, BA_ID:BA_ID + 128]
            w18_sb = blobA_sb[:, BA_W18:BA_W18 + 18]
            convb2_sb = blobA_sb[:, BA_CONVB:BA_CONVB + 2]
            qkvwT_sb = const.tile([128, 2, 3 * C], F32R, tag="qkvwT")
            for ct in range(2):
                nc.sync.dma_start(
                    qkvwT_sb[:, ct, 0:512],
                    qkvwT_d[ct * 128:(ct + 1) * 128, 0:512],
                )
            nc.sync.dma_start(
                qkvwT_sb[:, :, 512:768],
                qkvwT_d[:, 512:768].rearrange("(kc p) f -> p kc f", p=128),
            )
            blobB_sb = const.tile([128, BBW], BF16, tag="blobB")
            nc.sync.dma_start(blobB_sb, blobB_d)
            outwT_sb = blobB_sb[:, BB_OWT:BB_OWT + 512].rearrange(
                "p (kc f) -> p kc f", kc=2)
            outb_sb = blobB_sb[0:1, BB_OUTB:BB_OUTB + 256]

            # diag conv matrices: diag[c, t, f] = w18[c, t] * id[c, f] via
            # one DVE tensor_tensor per channel tile (DVE is idle at startup
            # and this beats the Pool affine_select by ~3us of latency)
            # ---- warm-ups (after the DMA issues so they don't block the
            # ACT queue): the exp ACT-table load and a chained trickle of
            # tiny PE matmuls (keeps the PE "recently active" through the
            # DMA wait so the conv burst is not dispatched into the cost
            # model's cold p-state)
            zerob_sb = const.tile([128, 1], F32, tag="zerob")
            nc.vector.memset(zerob_sb, 0.0)
            warm_sb = const.tile([1, 1], F32, tag="warm")
            nc.scalar.activation(
                warm_sb, zerob_sb[0:1, 0:1], AF.Exp,
                bias=zerob_sb[0:1], scale=1.0,
            )
            wv = const.tile([1, 20], F32, tag="wv")
            nc.vector.memset(wv, 0.0)
            for k in range(17):
                wps = pst.tile([128, 2, 512], F32, tag="ps", name="wps")
                nc.tensor.matmul(
                    wps[0:1, 0, 0:1], lhsT=wv[0:1, k:k + 1],
                    rhs=wv[0:1, k:k + 1], start=True, stop=True,
                )
                if k + 1 < 20:
                    nc.scalar.copy(wv[0:1, k + 1:k + 2], wps[0:1, 0, 0:1])

            diag_sb = const.tile([128, 18, 128], BF16, tag="diag")

            def emit_diag(ct):
                idb = bass.AP(
                    tensor=id_sb.tensor, offset=id_sb.offset,
                    ap=[list(id_sb.ap[0]), [0, 9], [1, 128]],
                )
                w18b = bass.AP(
                    tensor=w18_sb.tensor,
                    offset=w18_sb.offset + ct * 9,
                    ap=[list(w18_sb.ap[0]), [1, 9], [0, 128]],
                )
                nc.vector.tensor_tensor(
                    out=diag_sb[:, ct * 9:(ct + 1) * 9, :],
                    in0=idb, in1=w18b, op=ALU.mult,
                )

            def emit_xpad(ct):
                nc.vector.tensor_copy(
                    xpadT[:, ct, :].rearrange("p (h w) -> p h w", h=PAD)[
                        :, 1:33, 1:33
                    ],
                    xstg[:, ct, :].rearrange("p (h w) -> p h w", h=32),
                )

            # ones row generated on device (proj-bias rhs)
            ones_sb = const.tile([1, 512], BF16, tag="ones")
            nc.gpsimd.memset(ones_sb, 1.0)
            # conv bias in f32 for the per-partition bias of the conv
            # evacuation (folds the bias add into the PSUM->SBUF copy)
            convbf = const.tile([128, 2], F32, tag="convbf")
            # DVE order matters: ct0's conv inputs complete before ct1's
            # begin, so the ct0 conv matmuls start ~2us sooner
            nc.vector.tensor_copy(convbf, convb2_sb)
            emit_diag(0)
            emit_xpad(0)
            emit_diag(1)
            emit_xpad(1)

            yT = big.tile([128, 2, N], F32R, tag="yT")
            qT = big.tile([128, 2, N], F32R, tag="qT")
            kT = big.tile([128, 2, N], F32R, tag="kT")
            # [v_h | 1] per (token-chunk, head); ones preset via memset
            vsb = big.tile([128, 8, 8 * 33], BF16, tag="v")
            nc.gpsimd.memset(vsb, 1.0)
            a_sb = big.tile([128, 8, 256], BF16, tag="a_sb")
            attnT = big.tile([128, 2, N], BF16, tag="attnT")
            partial1 = big.tile([128, 8, C], F32R, tag="partial1")
            # f32r identity: lets the tail fold `+ partial1` into the
            # projection PSUM group as a K=128 matmul instead of a DVE add
            idr_sb = const.tile([128, 128], F32R, tag="idr")

            # psum evacuations: GPSIMD cannot access PSUM on HW, so they
            # alternate between the ACT (scalar.copy) and DVE engines
            _cp = [0]

            def copy_alt(dst, src_ap):
                _cp[0] += 1
                if _cp[0] % 2:
                    nc.scalar.copy(dst, src_ap)
                else:
                    nc.vector.tensor_copy(dst, src_ap)

            # ---- conv: per (ct, j) half: 9 diagonal matmuls + K=1 bias tap,
            # one 512-wide evacuation (j-split so the attention wavefront can
            # start on the j0 token half while j1 is still convolving)
            def emit_conv_half(ct, j):
                cps = pst.tile([128, 512], F32, tag="ps", name=f"cacc{ct}{j}")
                view = xpadT[:, ct, :].rearrange("p (h w) -> p h w", h=PAD)
                for t, (ky, kx) in enumerate(TAPS):
                    nc.tensor.matmul(
                        cps,
                        lhsT=diag_sb[:, ct * 9 + t, :],
                        rhs=view[:, ky + 16 * j: ky + 16 * j + 16, kx: kx + 32],
                        start=(t == 0),
                        stop=(t == 8),
                    )
                # conv bias folded into the evacuation (per-partition add)
                dst = yT[:, ct, j * 512:(j + 1) * 512]
                _cp[0] += 1
                if _cp[0] % 2:
                    nc.scalar.activation(
                        dst, cps, AF.Identity,
                        bias=convbf[:, ct:ct + 1], scale=1.0)
                else:
                    cb = bass.AP(
                        tensor=convbf.tensor, offset=convbf.offset + ct,
                        ap=[list(convbf.ap[0]), [0, 512]],
                    )
                    nc.vector.tensor_tensor(
                        out=dst, in0=cps, in1=cb, op=ALU.add)

            # ---- q^T / k^T: full feature tile or single token-half ----
            def emit_qk_half(ft, j):
                # in-loop extra: evacuation pinned to ACT (DVE is the
                # loop's ceiling engine)
                dstT, dc = (qT, ft) if ft < 2 else (kT, ft - 2)
                fofs = 0 if ft < 2 else 256
                qps = pst.tile([128, 512], F32, tag="ps", name="qps")
                for kc in range(2):
                    nc.tensor.matmul(
                        qps,
                        lhsT=qkvwT_sb[:, kc, fofs + dc * 128: fofs + (dc + 1) * 128],
                        rhs=yT[:, kc, j * 512:(j + 1) * 512],
                        start=(kc == 0),
                        stop=(kc == 1),
                    )
                nc.scalar.copy(dstT[:, dc, j * 512:(j + 1) * 512], qps)

            def emit_qk(ft):
                dstT, dc = (qT, ft) if ft < 2 else (kT, ft - 2)
                fofs = 0 if ft < 2 else 256
                qps = pst.tile([128, 2, 512], F32, tag="ps", name="qps")
                for j in range(2):
                    for kc in range(2):
                        nc.tensor.matmul(
                            qps[:, j, :],
                            lhsT=qkvwT_sb[:, kc, fofs + dc * 128: fofs + (dc + 1) * 128],
                            rhs=yT[:, kc, j * 512:(j + 1) * 512],
                            start=(kc == 0),
                            stop=(kc == 1),
                        )
                copy_alt(dstT[:, dc, :], qps.rearrange("p a b -> p (a b)"))

            # ---- v: 4 token chunks per unit, 8 matmuls, one strided evac ----
            def emit_v4(u):
                vps = pst.tile([128, 2, 512], F32, tag="ps", name="vps")
                for q in range(4):
                    nt = u * 4 + q
                    dst = vps[:, q // 2, (q % 2) * 256:(q % 2) * 256 + 256]
                    for kc in range(2):
                        # one open accumulation group per bank: start on the
                        # bank's first write, stop on its last
                        nc.tensor.matmul(
                            dst,
                            lhsT=yT[:, kc, nt * 128:(nt + 1) * 128],
                            rhs=qkvwT_sb[:, kc, 512:768],
                            start=(kc == 0 and q % 2 == 0),
                            stop=(kc == 1 and q % 2 == 1),
                        )
                sv = vps.rearrange("p a (q hh c) -> p (a q) hh c", q=2, c=32)
                dv = vsb[:, u * 4:(u + 1) * 4, :].rearrange(
                    "p n (hh c) -> p n hh c", c=33)[:, :, :, 0:32]
                if u == 1:
                    # in-loop extra: pin to ACT
                    nc.scalar.copy(dv, sv)
                else:
                    copy_alt(dv, sv)

            # pre-loop: exactly what pair 0 needs up front (chunk-1 q/k and
            # the first four v chunks); the rest trickles in as one light
            # half-unit extra per m-step so the S/exp PSUM rotation is never
            # starved for long
            for ct in range(2):
                for j in range(2):
                    emit_conv_half(ct, j)
            emit_qk(1)
            emit_qk(3)
            emit_v4(0)

            # ---- a_sb -> attnT: 8 transposes sharing one bank + ONE copy ----
            def emit_atr_mm(ct, nc_i, tp):
                nc.tensor.matmul(
                    tp[:, nc_i * 128:(nc_i + 1) * 128],
                    lhsT=a_sb[:, nc_i, ct * 128:(ct + 1) * 128],
                    rhs=id_sb,
                    is_transpose=True,
                    start=(nc_i == 0),
                    stop=(nc_i == 7),
                )

            def emit_proj1_pair(np_):
                # token chunks (2*np_, 2*np_+1) share one bank; lazy
                # region-zeroing from the first start covers the second group.
                pj = pst.tile([128, 512], F32, tag="ps", name="pj1")
                for q in range(2):
                    nt = np_ * 2 + q
                    dst = pj[:, q * 256:(q + 1) * 256]
                    nc.tensor.matmul(
                        dst,
                        lhsT=attnT[:, 1, nt * 128:(nt + 1) * 128],
                        rhs=outwT_sb[:, 1, :],
                        start=(q == 0),
                        stop=False,
                    )
                    nc.tensor.matmul(
                        dst,
                        lhsT=ones_sb[0:1, 0:128],
                        rhs=outb_sb,
                        start=False,
                        stop=(q == 1),
                    )
                # ACT: the in-loop DVE queue carries the hs1 exps (2-step
                # slack) near saturation; ACT has headroom between hs0 exps
                nc.scalar.copy(
                    partial1[:, np_ * 2:(np_ + 1) * 2, :].rearrange(
                        "p a b -> p (a b)"),
                    pj,
                )

            # interleaved extras, one self-contained slice per m-step
            def emit_atr_ct(ct):
                # all 8 transposes share one bank-tile + ONE wide copy on
                # ACT (DVE is the loop's ceiling engine); single slice keeps
                # the PSUM slot hold under ~1 m-step
                tp = pst.tile([128, 1024], BF16, tag="ps", name=f"atp{ct}")
                for i in range(8):
                    emit_atr_mm(ct, i, tp)
                nc.scalar.copy(attnT[:, ct, :], tp)

            def pair_extra(ip, m):
                if ip == 0:
                    if m == 1:
                        emit_v4(1)
                    elif m == 3:
                        emit_qk_half(0, 0)
                    elif m == 5:
                        emit_qk_half(0, 1)
                elif ip == 1:
                    if m == 1:
                        emit_qk_half(2, 0)
                    elif m == 3:
                        emit_qk_half(2, 1)
                    elif m == 4:
                        # SBUF->SBUF: runs on the otherwise-idle Pool engine
                        nc.gpsimd.tensor_copy(idr_sb, id_sb)
                elif ip == 2:
                    if m == 6:
                        emit_atr_ct(1)
                elif ip == 3:
                    if m == 1:
                        emit_proj1_pair(0)
                    elif m == 3:
                        emit_proj1_pair(1)
                    elif m == 5:
                        emit_proj1_pair(2)

            # ---- merged exp: ONE 1024-wide instruction per (head, m).
            # hs0 sits on the 1-step-slack PSUM slot: its exp gates the
            # S-issue chain, so it always runs on the faster ACT engine.
            # hs1 (2-step slack) goes to DVE except two steps per pair,
            # balancing total engine busy (~42 ACT / 22 DVE tiles).
            def emit_exp_half(eng, sv, w):
                if eng == "A":
                    p = ppool.tile([128, w], BF16, tag="pT", name="pA")
                    nc.scalar.activation(p, sv, AF.Exp, bias=zerob_sb, scale=SCALE)
                    return p
                p = ppool.tile([128, w], I16, tag="pT", name="pV")
                nc.vector.tensor_scalar(
                    out=p, in0=sv, scalar1=SCHR_A, scalar2=SCHR_B,
                    op0=ALU.mult, op1=ALU.add,
                )
                return p.bitcast(BF16)

            def emit_exp(eng, st2):
                return emit_exp_half(
                    eng, st2.rearrange("p a b -> p (a b)"), 1024)

            # ---- attention ----
            def emit_pv(m, ph, pas, heads, rng=None):
                # one accumulation group per pa bank: start only on the first
                # write (lazy 2KB region-zeroing covers the other 7
                # sub-regions), stop only on the last. rng selects a 4-chunk
                # n-range for the pair-0 wavefront half-tiles.
                base = 0 if rng is None else rng
                for nc_i in (range(8) if rng is None else range(rng, rng + 4)):
                    for hs in (0, 1):
                        nc.tensor.matmul(
                            pas[hs][:, nc_i * 33: nc_i * 33 + 33],
                            lhsT=ph[hs][:, (nc_i - base) * 128:
                                        (nc_i - base + 1) * 128],
                            rhs=vsb[:, m, 33 * heads[hs]: 33 * heads[hs] + 33],
                            start=(m == 0 and nc_i == 0),
                            stop=(m == 7 and nc_i == 7),
                        )

            def emit_norm(pas, heads):
                for h, pa in zip(heads, pas):
                    pav = pa.rearrange("p (nc e) -> p nc e", e=33)
                    rcp = rcp_p.tile([128, 8], F32, tag="rcp", name="rcp")
                    nc.vector.reciprocal(rcp, pav[:, :, 32])
                    rcp_b = bass.AP(
                        tensor=rcp.tensor, offset=rcp.offset,
                        ap=[list(rcp.ap[0]), [1, 8], [0, 32]],
                    )
                    nc.vector.tensor_tensor(
                        out=a_sb[:, :, h * 32: h * 32 + 32],
                        in0=pav[:, :, 0:32],
                        in1=rcp_b,
                        op=ALU.mult,
                    )

            def emit_s_half(h, m, j):
                a = 32 * (h % 4)
                hc = h // 4
                sth = pst.tile([128, 512], F32, tag="ps", name="sth")
                nc.tensor.matmul(
                    sth,
                    lhsT=kT[a:a + 32, hc, m * 128:(m + 1) * 128],
                    rhs=qT[a:a + 32, hc, j * 512:(j + 1) * 512],
                    start=True,
                    stop=True,
                    tile_position=(a, 0),
                )
                return sth

            def emit_s_full(h, m):
                a = 32 * (h % 4)
                hc = h // 4
                st2 = pst.tile([128, 2, 512], F32, tag="ps", name="st")
                for j in range(2):
                    nc.tensor.matmul(
                        st2[:, j, :],
                        lhsT=kT[a:a + 32, hc, m * 128:(m + 1) * 128],
                        rhs=qT[a:a + 32, hc, j * 512:(j + 1) * 512],
                        start=True,
                        stop=True,
                        tile_position=(a, 0),
                    )
                return st2

            carry = []
            for ip, (hA, hB) in enumerate(PAIRS):
                pas = (
                    pap.tile([128, 264], F32, tag="pa", name=f"paA{ip}"),
                    pap.tile([128, 264], F32, tag="pa", name=f"paB{ip}"),
                )
                heads = (hA, hB)
                pend = []
                for m in range(8):
                    ph = {
                        hs: emit_exp(
                            "A" if hs == 0 else "V", emit_s_full(h, m))
                        for hs, h in ((0, hA), (1, hB))
                    }
                    pend.append((m, ph, None))
                    # carried PVs wait until m>=2 so the previous pair's
                    # trailing exps (still draining on DVE) don't head-of-line
                    # stall the PE queue
                    if carry and m >= 2:
                        carry.pop(0)()
                    pair_extra(ip, m)
                    # the last pair drains its PVs earlier to shorten the tail
                    if len(pend) > (1 if ip == 3 else 2):
                        e = pend.pop(0)
                        emit_pv(e[0], e[1], pas, heads, e[2])
                # defer the tail PVs + normalization into the next pair's
                # m-loop so the PE never waits on the trailing exps
                thunks = [
                    (lambda e=e, pas=pas, heads=heads: emit_pv(
                        e[0], e[1], pas, heads, e[2]))
                    for e in pend
                ]
                for hs in (0, 1):
                    thunks.append(
                        lambda hs=hs, pas=pas, heads=heads: emit_norm(
                            (pas[hs],), (heads[hs],))
                    )
                carry = thunks

            # ---- tail: last pair's PVs + norms first (they gate the whole
            # output chain), then the remaining chunk-1 projection ----
            for t in carry:  # PV(7) j-halves, the two norms
                t()
            emit_proj1_pair(3)

            if debug_dump:
                nc.sync.dma_start(dbg["d_yT"], yT.bitcast(F32))
                nc.sync.dma_start(dbg["d_qT"], qT.bitcast(F32))
                nc.sync.dma_start(dbg["d_kT"], kT.bitcast(F32))
                dvf = big.tile([128, 8, 264], F32, tag="dvf")
                nc.vector.tensor_copy(dvf, vsb)
                nc.sync.dma_start(dbg["d_v"], dvf)
                daf = big.tile([128, 8, 256], F32, tag="daf")
                nc.vector.tensor_copy(daf, a_sb)
                nc.sync.dma_start(dbg["d_asb"], daf)

            # transpose chunk-0 (shared-bank, half-copies so the first
            # projections start before the second half lands), project in
            # token-chunk pairs, re-add staged half via identity matmul,
            # merged copies, store
            tp0 = pst.tile([128, 1024], BF16, tag="ps", name="atp0")
            for i in range(4):
                emit_atr_mm(0, i, tp0)
            for i in range(4, 8):
                emit_atr_mm(0, i, tp0)
            nc.vector.tensor_copy(attnT[:, 0, 0:512], tp0[:, 0:512])
            nc.vector.tensor_copy(attnT[:, 0, 512:1024], tp0[:, 512:1024])
            for np_ in range(4):
                ops = pst.tile([128, 2, 512], F32, tag="ps", name="ops")
                for q in range(2):
                    nt = np_ * 2 + q
                    dst = ops[:, 0, q * 256:(q + 1) * 256]
                    nc.tensor.matmul(
                        dst,
                        lhsT=attnT[:, 0, nt * 128:(nt + 1) * 128],
                        rhs=outwT_sb[:, 0, :],
                        start=(q == 0),
                        stop=False,
                    )
                    nc.tensor.matmul(
                        dst,
                        lhsT=idr_sb,
                        rhs=partial1[:, nt, :],
                        start=False,
                        stop=(q == 1),
                    )
                osb2 = outs_p.tile([128, 2, C], F32, tag="o", name="osb2")
                # alternate engines: DVE is idle once the last norms are done
                copy_alt(osb2.rearrange("p a b -> p (a b)"), ops[:, 0, :])
                # one batched DMA per 2 token chunks (HWDGE overhead is
                # per-descriptor-set, ~625ns each)
                oq = nc.sync if np_ % 2 else nc.scalar
                oq.dma_start(
                    out_d[np_ * 256:(np_ + 1) * 256, :].rearrange(
                        "(c p) f -> p c f", p=128),
                    osb2,
                )

    nc.compile()
    return nc


_NC = None
LAST_RESULTS = None


def _host_prep(conv_w, conv_b, qkv_w, out_w, out_b):
    import ml_dtypes

    conv_w = np.asarray(conv_w, np.float32).reshape(C, 3, 3)
    w18 = np.zeros((128, 18), np.float32)
    for ct in range(2):
        for t, (ky, kx) in enumerate(TAPS):
            d = conv_w[128 * ct: 128 * (ct + 1), ky, kx].copy()
            if (ky, kx) == (1, 1):
                d += 1.0  # residual connection folded into the center tap
            w18[:, ct * 9 + t] = d
    blobA = np.zeros((128, BAW), ml_dtypes.bfloat16)
    blobA[:, BA_ID:BA_ID + 128] = np.eye(128, dtype=ml_dtypes.bfloat16)
    blobA[:, BA_W18:BA_W18 + 18] = w18.astype(ml_dtypes.bfloat16)
    cb = np.asarray(conv_b, np.float32).reshape(2, 128).T
    blobA[:, BA_CONVB:BA_CONVB + 2] = cb.astype(ml_dtypes.bfloat16)
    blobB = np.zeros((128, BBW), ml_dtypes.bfloat16)
    owT = np.ascontiguousarray(np.asarray(out_w, np.float32).T).astype(
        ml_dtypes.bfloat16)  # [256 in, 256 outc]
    blobB[:, BB_OWT:BB_OWT + 512] = np.concatenate(
        [owT[0:128, :], owT[128:256, :]], axis=1)
    blobB[0, BB_OUTB:BB_OUTB + 256] = np.asarray(out_b, np.float32).astype(
        ml_dtypes.bfloat16)
    return {
        "qkv_wT": np.ascontiguousarray(np.asarray(qkv_w, np.float32).T),
        "blobA": blobA,
        "blobB": blobB,
    }


def _prep_x(x):
    """bf16, host-transposed to [B, C, N] for straight (transpose-free) DMA."""
    import ml_dtypes

    xt = np.swapaxes(np.asarray(x, np.float32), -1, -2)
    return np.ascontiguousarray(xt.astype(ml_dtypes.bfloat16))


def kernel(x, conv_w, conv_b, qkv_w, out_w, out_b):
    global _NC, LAST_RESULTS

    if _NC is None:
        _NC = build_nc()
    x = _prep_x(x)
    shared = _host_prep(conv_w, conv_b, qkv_w, out_w, out_b)
    in_maps = [{**shared, "x": np.ascontiguousarray(x[b])} for b in range(B)]
    trace = bool(int(os.environ.get("KERNEL_TRACE", "0")))
    try:
        res = run_bass_kernel_spmd(_NC, in_maps, core_ids=list(range(B)), trace=trace)
    except Exception:
        if not trace:
            raise
        res = run_bass_kernel_spmd(_NC, in_maps, core_ids=list(range(B)), trace=False)
    LAST_RESULTS = res
    return np.stack([res.results[b]["out"] for b in range(B)], axis=0)


# revision 74
# speedup vs baseline: 1.0117x; 1.0117x over previous
"""Trainium2 Bass kernel for nn_Attention_43190191129190.

Model (per batch element b of 8):
    y   = x + dwconv3x3(x) + conv_b          (depthwise residual positional conv)
    qkv = y @ qkv_w.T ; split into q, k, v   (8 heads, dim 32)
    out = softmax(q k^T / sqrt(32)) v
    out = out @ out_w.T + out_b
Sharding: pure data-parallel, one batch element per NeuronCore (8 cores).

Per-core design (v5 — merged 2-bank tiles, wide exp, merged evacuations):

  The ACT+DVE engines are the wall: every S element must be exp'd
  (65536 cols of [128]-partition work) and every PSUM result must be
  evacuated by ACT/DVE (DMA and GPSIMD cannot touch PSUM). v5 cuts the
  per-instruction init overhead (ACT ~185ns, DVE ~125ns busy per op) by
  merging work into the widest possible instructions:

  1. x arrives bf16, host pre-transposed; 2 DMA-xbar transposes stage
     x^T, copied into a zero-haloed [C, 34, 34] image.
  2. diag conv matrices built on device by TWO [128,9,128]
     affine_selects (one per channel tile) instead of 18 narrow ones.
  3. conv per ct: one [128,2,512] PSUM tile, j halves as two 10-matmul
     accumulation groups, ONE [128,1024] evacuation (bias via K=1 tap).
  4. q^T/k^T per feature tile: one [128,2,512] tile, 4 matmuls, ONE
     evacuation. v: two 4-token-chunk units, 8 matmuls + ONE strided
     evacuation each into [v_h|1] 33-wide head slots (ones preset).
  5. Attention, head pair per generation, 8 m-steps each:
       S^T per (head, m): one [128,2,512] f32 PSUM tile (two 512-wide
       matmuls), then ONE 1024-wide exp:
         hs0 -> ACT exact Exp -> bf16; hs1 -> DVE Schraudolph
         (tensor_scalar s*A+B -> int16 bits == bf16(exp(s*SCALE))).
       PV unchanged: per-head [128, 8x33] PSUM accumulator, stationary
       p^T chunks, moving [v_h|1]; column 32 = softmax denominators;
       one accumulation group per bank. Norm per head: reciprocal +
       one broadcast tensor_tensor -> a_sb bf16.
  6. a_sb -> attnT: 8 transposes per ct share ONE [128,1024] bf16 bank
     (single accumulation group, disjoint regions), ONE 2x-mode DVE
     copy per ct.
  7. projection: chunk-1 + out_b staged mid-kernel into partial1 (pairs
     of token chunks share a bank, one [128,512] copy); tail re-adds
     partial1 via f32r identity matmuls and stores via merged copies.

  PSUM: 3x[128,2,512] f32 rotating slots + 2x[128,264] PV accumulators.
  Pre-attention work interleaves into the pair loops one self-contained
  slice per m-step (alloc+use+evacuate within the slice).
"""

import os

import numpy as np

import concourse.bass as bass
import concourse.tile as tile
from concourse import bacc, mybir
from concourse.bass_utils import run_bass_kernel_spmd

F32 = mybir.dt.float32
F32R = mybir.dt.float32r
BF16 = mybir.dt.bfloat16
I16 = mybir.dt.int16
AF = mybir.ActivationFunctionType
ALU = mybir.AluOpType

B, N, C = 8, 1024, 256
HEADS, DH = 8, 32
SCALE = DH ** -0.5
PAD = 34  # 32x32 spatial grid with 1-px halo

# blobA (bf16): id [128, 0:128] | w18 [128, 128:146] | convb cols [128, 146:148]
BA_ID, BA_W18, BA_CONVB, BAW = 0, 128, 146, 148
# blobB (bf16): outwT [128, 0:512] | outb row0 [512:768]
BB_OWT, BB_OUTB, BBW = 0, 512, 768

TAPS = [(ky, kx) for ky in range(3) for kx in range(3)]
# chunk-1 head pairs first so the chunk-1 projection can run mid-kernel;
# the tail then only waits on the last pair's (chunk-0) normalization
PAIRS = [(5, 7), (4, 6), (1, 3), (0, 2)]

# Schraudolph fast-exp: int16 bits of bf16(exp(s*SCALE)) = s*A + B
SCHR_C = 450000.0
SCHR_A = float(SCALE * (2 ** 23) / np.log(2) / 65536.0)
SCHR_B = float((127 * 2 ** 23 - SCHR_C) / 65536.0)


def build_nc(debug_dump=False):
    nc = bacc.Bacc("TRN2", target_bir_lowering=False, debug=False, num_devices=8)

    # x arrives host-pre-transposed: [C, N] bf16, one straight DMA
    x_d = nc.dram_tensor("x", (C, N), BF16, kind="ExternalInput").ap()
    qkvwT_d = nc.dram_tensor("qkv_wT", (C, 3 * C), F32R, kind="ExternalInput").ap()
    blobA_d = nc.dram_tensor("blobA", (128, BAW), BF16, kind="ExternalInput").ap()
    blobB_d = nc.dram_tensor("blobB", (128, BBW), BF16, kind="ExternalInput").ap()
    out_d = nc.dram_tensor("out", (N, C), F32, kind="ExternalOutput").ap()
    dbg = {}
    if debug_dump:
        for name, shape in (
            ("d_yT", (128, 2, N)), ("d_qT", (128, 2, N)), ("d_kT", (128, 2, N)),
            ("d_v", (128, 8, 264)), ("d_asb", (128, 8, 256)),
        ):
            dbg[name] = nc.dram_tensor(name, shape, F32, kind="ExternalOutput").ap()

    with tile.TileContext(nc) as tc:
        with (
            tc.tile_pool(name="const", bufs=1) as const,
            tc.tile_pool(name="big", bufs=1) as big,
            tc.tile_pool(name="pT", bufs=16) as ppool,
            tc.tile_pool(name="rcp", bufs=4) as rcp_p,
            tc.tile_pool(name="outs", bufs=4) as outs_p,
            tc.tile_pool(name="pst", bufs=3, space="PSUM") as pst,
            tc.tile_pool(name="pap", bufs=2, space="PSUM") as pap,
        ):
            # ---- persistent activations (x image first: DMA critical path)
            xpadT = big.tile([128, 2, PAD * PAD], BF16, tag="xpadT")
            xpv = xpadT.bitcast(mybir.dt.uint16).rearrange(
                "p ct (h w) -> p ct h w", h=PAD
            )
            nc.vector.memset(xpv[:, :, 0, :], 0)
            nc.vector.memset(xpv[:, :, PAD - 1, :], 0)
            nc.vector.memset(xpv[:, :, :, 0], 0)
            nc.vector.memset(xpv[:, :, :, PAD - 1], 0)

            # ---- DMAs. Per-DMA cost in the serial DMA pipeline is large
            # (HWDGE 625 + DGE delay 650 + transfer + completion sem 900),
            # so x is host-pre-transposed and lands in ONE straight DMA.
            blobA_sb = const.tile([128, BAW], BF16, tag="blobA")
            nc.sync.dma_start(blobA_sb, blobA_d)
            xstg = big.tile([128, 2, N], BF16, tag="xstg")
            for ct in range(2):
                nc.sync.dma_start(xstg[:, ct, :],
                                  x_d[ct * 128:(ct + 1) * 128, :])
            id_sb = blobA_sb[:, BA_ID:BA_ID + 128]
            w18_sb = blobA_sb[:, BA_W18:BA_W18 + 18]
            convb2_sb = blobA_sb[:, BA_CONVB:BA_CONVB + 2]
            qkvwT_sb = const.tile([128, 2, 3 * C], F32R, tag="qkvwT")
            for ct in range(2):
                nc.sync.dma_start(
                    qkvwT_sb[:, ct, 0:512],
                    qkvwT_d[ct * 128:(ct + 1) * 128, 0:512],
                )
            nc.sync.dma_start(
                qkvwT_sb[:, :, 512:768],
                qkvwT_d[:, 512:768].rearrange("(kc p) f -> p kc f", p=128),
            )
            blobB_sb = const.tile([128, BBW], BF16, tag="blobB")
            nc.sync.dma_start(blobB_sb, blobB_d)
            outwT_sb = blobB_sb[:, BB_OWT:BB_OWT + 512].rearrange(
                "p (kc f) -> p kc f", kc=2)
            outb_sb = blobB_sb[0:1, BB_OUTB:BB_OUTB + 256]

            # diag conv matrices: diag[c, t, f] = w18[c, t] * id[c, f] via
            # one DVE tensor_tensor per channel tile (DVE is idle at startup
            # and this beats the Pool affine_select by ~3us of latency)
            # ---- warm-ups (after the DMA issues so they don't block the
            # ACT queue): the exp ACT-table load and a chained trickle of
            # tiny PE matmuls (keeps the PE "recently active" through the
            # DMA wait so the conv burst is not dispatched into the cost
            # model's cold p-state)
            zerob_sb = const.tile([128, 1], F32, tag="zerob")
            nc.vector.memset(zerob_sb, 0.0)
            warm_sb = const.tile([1, 1], F32, tag="warm")
            nc.scalar.activation(
                warm_sb, zerob_sb[0:1, 0:1], AF.Exp,
                bias=zerob_sb[0:1], scale=1.0,
            )
            wv = const.tile([1, 20], F32, tag="wv")
            nc.vector.memset(wv, 0.0)
            for k in range(17):
                wps = pst.tile([128, 2, 512], F32, tag="ps", name="wps")
                nc.tensor.matmul(
                    wps[0:1, 0, 0:1], lhsT=wv[0:1, k:k + 1],
                    rhs=wv[0:1, k:k + 1], start=True, stop=True,
                )
                if k + 1 < 20:
                    nc.scalar.copy(wv[0:1, k + 1:k + 2], wps[0:1, 0, 0:1])

            diag_sb = const.tile([128, 18, 128], BF16, tag="diag")

            def emit_diag(ct):
                idb = bass.AP(
                    tensor=id_sb.tensor, offset=id_sb.offset,
                    ap=[list(id_sb.ap[0]), [0, 9], [1, 128]],
                )
                w18b = bass.AP(
                    tensor=w18_sb.tensor,
                    offset=w18_sb.offset + ct * 9,
                    ap=[list(w18_sb.ap[0]), [1, 9], [0, 128]],
                )
                nc.vector.tensor_tensor(
                    out=diag_sb[:, ct * 9:(ct + 1) * 9, :],
                    in0=idb, in1=w18b, op=ALU.mult,
                )

            def emit_xpad(ct):
                nc.vector.tensor_copy(
                    xpadT[:, ct, :].rearrange("p (h w) -> p h w", h=PAD)[
                        :, 1:33, 1:33
                    ],
                    xstg[:, ct, :].rearrange("p (h w) -> p h w", h=32),
                )

            # ones row generated on device (proj-bias rhs)
            ones_sb = const.tile([1, 512], BF16, tag="ones")
            nc.gpsimd.memset(ones_sb, 1.0)
            # conv bias in f32 for the per-partition bias of the conv
            # evacuation (folds the bias add into the PSUM->SBUF copy)
            convbf = const.tile([128, 2], F32, tag="convbf")
            # DVE order matters: ct0's conv inputs complete before ct1's
            # begin, so the ct0 conv matmuls start ~2us sooner
            nc.vector.tensor_copy(convbf, convb2_sb)
            emit_diag(0)
            emit_xpad(0)
            emit_diag(1)
            emit_xpad(1)

            yT = big.tile([128, 2, N], F32R, tag="yT")
            qT = big.tile([128, 2, N], F32R, tag="qT")
            kT = big.tile([128, 2, N], F32R, tag="kT")
            # [v_h | 1] per (token-chunk, head); ones preset via memset
            vsb = big.tile([128, 8, 8 * 33], BF16, tag="v")
            nc.gpsimd.memset(vsb, 1.0)
            a_sb = big.tile([128, 8, 256], BF16, tag="a_sb")
            attnT = big.tile([128, 2, N], BF16, tag="attnT")
            partial1 = big.tile([128, 8, C], F32R, tag="partial1")
            # f32r identity: lets the tail fold `+ partial1` into the
            # projection PSUM group as a K=128 matmul instead of a DVE add
            idr_sb = const.tile([128, 128], F32R, tag="idr")

            # psum evacuations: GPSIMD cannot access PSUM on HW, so they
            # alternate between the ACT (scalar.copy) and DVE engines
            _cp = [0]

            def copy_alt(dst, src_ap):
                _cp[0] += 1
                if _cp[0] % 2:
                    nc.scalar.copy(dst, src_ap)
                else:
                    nc.vector.tensor_copy(dst, src_ap)

            # ---- conv: per (ct, j) half: 9 diagonal matmuls + K=1 bias tap,
            # one 512-wide evacuation (j-split so the attention wavefront can
            # start on the j0 token half while j1 is still convolving)
            def emit_conv_half(ct, j):
                cps = pst.tile([128, 512], F32, tag="ps", name=f"cacc{ct}{j}")
                view = xpadT[:, ct, :].rearrange("p (h w) -> p h w", h=PAD)
                for t, (ky, kx) in enumerate(TAPS):
                    nc.tensor.matmul(
                        cps,
                        lhsT=diag_sb[:, ct * 9 + t, :],
                        rhs=view[:, ky + 16 * j: ky + 16 * j + 16, kx: kx + 32],
                        start=(t == 0),
                        stop=(t == 8),
                    )
                # conv bias folded into the evacuation (per-partition add)
                dst = yT[:, ct, j * 512:(j + 1) * 512]
                _cp[0] += 1
                if _cp[0] % 2:
                    nc.scalar.activation(
                        dst, cps, AF.Identity,
                        bias=convbf[:, ct:ct + 1], scale=1.0)
                else:
                    cb = bass.AP(
                        tensor=convbf.tensor, offset=convbf.offset + ct,
                        ap=[list(convbf.ap[0]), [0, 512]],
                    )
                    nc.vector.tensor_tensor(
                        out=dst, in0=cps, in1=cb, op=ALU.add)

            # ---- q^T / k^T: full feature tile or single token-half ----
            def emit_qk_half(ft, j):
                dstT, dc = (qT, ft) if ft < 2 else (kT, ft - 2)
                fofs = 0 if ft < 2 else 256
                qps = pst.tile([128, 512], F32, tag="ps", name="qps")
                for kc in range(2):
                    nc.tensor.matmul(
                        qps,
                        lhsT=qkvwT_sb[:, kc, fofs + dc * 128: fofs + (dc + 1) * 128],
                        rhs=yT[:, kc, j * 512:(j + 1) * 512],
                        start=(kc == 0),
                        stop=(kc == 1),
                    )
                copy_alt(dstT[:, dc, j * 512:(j + 1) * 512], qps)

            def emit_qk(ft):
                dstT, dc = (qT, ft) if ft < 2 else (kT, ft - 2)
                fofs = 0 if ft < 2 else 256
                qps = pst.tile([128, 2, 512], F32, tag="ps", name="qps")
                for j in range(2):
                    for kc in range(2):
                        nc.tensor.matmul(
                            qps[:, j, :],
                            lhsT=qkvwT_sb[:, kc, fofs + dc * 128: fofs + (dc + 1) * 128],
                            rhs=yT[:, kc, j * 512:(j + 1) * 512],
                            start=(kc == 0),
                            stop=(kc == 1),
                        )
                copy_alt(dstT[:, dc, :], qps.rearrange("p a b -> p (a b)"))

            # ---- v: 4 token chunks per unit, 8 matmuls, one strided evac ----
            def emit_v4(u):
                vps = pst.tile([128, 2, 512], F32, tag="ps", name="vps")
                for q in range(4):
                    nt = u * 4 + q
                    dst = vps[:, q // 2, (q % 2) * 256:(q % 2) * 256 + 256]
                    for kc in range(2):
                        # one open accumulation group per bank: start on the
                        # bank's first write, stop on its last
                        nc.tensor.matmul(
                            dst,
                            lhsT=yT[:, kc, nt * 128:(nt + 1) * 128],
                            rhs=qkvwT_sb[:, kc, 512:768],
                            start=(kc == 0 and q % 2 == 0),
                            stop=(kc == 1 and q % 2 == 1),
                        )
                sv = vps.rearrange("p a (q hh c) -> p (a q) hh c", q=2, c=32)
                dv = vsb[:, u * 4:(u + 1) * 4, :].rearrange(
                    "p n (hh c) -> p n hh c", c=33)[:, :, :, 0:32]
                copy_alt(dv, sv)

            # pre-loop: exactly what pair 0 needs up front (chunk-1 q/k and
            # the first four v chunks); the rest trickles in as one light
            # half-unit extra per m-step so the S/exp PSUM rotation is never
            # starved for long
            for ct in range(2):
                for j in range(2):
                    emit_conv_half(ct, j)
            emit_qk(1)
            emit_qk(3)
            emit_v4(0)

            # ---- a_sb -> attnT: 8 transposes sharing one bank + ONE copy ----
            def emit_atr_mm(ct, nc_i, tp):
                nc.tensor.matmul(
                    tp[:, nc_i * 128:(nc_i + 1) * 128],
                    lhsT=a_sb[:, nc_i, ct * 128:(ct + 1) * 128],
                    rhs=id_sb,
                    is_transpose=True,
                    start=(nc_i == 0),
                    stop=(nc_i == 7),
                )

            def emit_proj1_pair(np_):
                # token chunks (2*np_, 2*np_+1) share one bank; lazy
                # region-zeroing from the first start covers the second group.
                pj = pst.tile([128, 512], F32, tag="ps", name="pj1")
                for q in range(2):
                    nt = np_ * 2 + q
                    dst = pj[:, q * 256:(q + 1) * 256]
                    nc.tensor.matmul(
                        dst,
                        lhsT=attnT[:, 1, nt * 128:(nt + 1) * 128],
                        rhs=outwT_sb[:, 1, :],
                        start=(q == 0),
                        stop=False,
                    )
                    nc.tensor.matmul(
                        dst,
                        lhsT=ones_sb[0:1, 0:128],
                        rhs=outb_sb,
                        start=False,
                        stop=(q == 1),
                    )
                # ACT: the in-loop DVE queue carries the hs1 exps (2-step
                # slack) near saturation; ACT has headroom between hs0 exps
                nc.scalar.copy(
                    partial1[:, np_ * 2:(np_ + 1) * 2, :].rearrange(
                        "p a b -> p (a b)"),
                    pj,
                )

            # interleaved extras, one self-contained slice per m-step
            def emit_atr_ct(ct):
                # all 8 transposes share one bank-tile + ONE 2x-mode copy;
                # single slice keeps the PSUM slot hold under ~1 m-step
                tp = pst.tile([128, 1024], BF16, tag="ps", name=f"atp{ct}")
                for i in range(8):
                    emit_atr_mm(ct, i, tp)
                nc.vector.tensor_copy(attnT[:, ct, :], tp)

            def pair_extra(ip, m):
                if ip == 0:
                    if m == 1:
                        emit_v4(1)
                    elif m == 3:
                        emit_qk_half(0, 0)
                    elif m == 5:
                        emit_qk_half(0, 1)
                elif ip == 1:
                    if m == 1:
                        emit_qk_half(2, 0)
                    elif m == 3:
                        emit_qk_half(2, 1)
                    elif m == 4:
                        # SBUF->SBUF: runs on the otherwise-idle Pool engine
                        nc.gpsimd.tensor_copy(idr_sb, id_sb)
                elif ip == 2:
                    if m == 6:
                        emit_atr_ct(1)
                elif ip == 3:
                    if m == 1:
                        emit_proj1_pair(0)
                    elif m == 3:
                        emit_proj1_pair(1)
                    elif m == 5:
                        emit_proj1_pair(2)

            # ---- merged exp: ONE 1024-wide instruction per (head, m).
            # hs0 sits on the 1-step-slack PSUM slot: its exp gates the
            # S-issue chain, so it always runs on the faster ACT engine.
            # hs1 (2-step slack) goes to DVE except two steps per pair,
            # balancing total engine busy (~42 ACT / 22 DVE tiles).
            def emit_exp_half(eng, sv, w):
                if eng == "A":
                    p = ppool.tile([128, w], BF16, tag="pT", name="pA")
                    nc.scalar.activation(p, sv, AF.Exp, bias=zerob_sb, scale=SCALE)
                    return p
                p = ppool.tile([128, w], I16, tag="pT", name="pV")
                nc.vector.tensor_scalar(
                    out=p, in0=sv, scalar1=SCHR_A, scalar2=SCHR_B,
                    op0=ALU.mult, op1=ALU.add,
                )
                return p.bitcast(BF16)

            def emit_exp(eng, st2):
                return emit_exp_half(
                    eng, st2.rearrange("p a b -> p (a b)"), 1024)

            # ---- attention ----
            def emit_pv(m, ph, pas, heads, rng=None):
                # one accumulation group per pa bank: start only on the first
                # write (lazy 2KB region-zeroing covers the other 7
                # sub-regions), stop only on the last. rng selects a 4-chunk
                # n-range for the pair-0 wavefront half-tiles.
                base = 0 if rng is None else rng
                for nc_i in (range(8) if rng is None else range(rng, rng + 4)):
                    for hs in (0, 1):
                        nc.tensor.matmul(
                            pas[hs][:, nc_i * 33: nc_i * 33 + 33],
                            lhsT=ph[hs][:, (nc_i - base) * 128:
                                        (nc_i - base + 1) * 128],
                            rhs=vsb[:, m, 33 * heads[hs]: 33 * heads[hs] + 33],
                            start=(m == 0 and nc_i == 0),
                            stop=(m == 7 and nc_i == 7),
                        )

            def emit_norm(pas, heads):
                for h, pa in zip(heads, pas):
                    pav = pa.rearrange("p (nc e) -> p nc e", e=33)
                    rcp = rcp_p.tile([128, 8], F32, tag="rcp", name="rcp")
                    nc.vector.reciprocal(rcp, pav[:, :, 32])
                    rcp_b = bass.AP(
                        tensor=rcp.tensor, offset=rcp.offset,
                        ap=[list(rcp.ap[0]), [1, 8], [0, 32]],
                    )
                    nc.vector.tensor_tensor(
                        out=a_sb[:, :, h * 32: h * 32 + 32],
                        in0=pav[:, :, 0:32],
                        in1=rcp_b,
                        op=ALU.mult,
                    )

            def emit_s_half(h, m, j):
                a = 32 * (h % 4)
                hc = h // 4
                sth = pst.tile([128, 512], F32, tag="ps", name="sth")
                nc.tensor.matmul(
                    sth,
                    lhsT=kT[a:a + 32, hc, m * 128:(m + 1) * 128],
                    rhs=qT[a:a + 32, hc, j * 512:(j + 1) * 512],
                    start=True,
                    stop=True,
                    tile_position=(a, 0),
                )
                return sth

            def emit_s_full(h, m):
                a = 32 * (h % 4)
                hc = h // 4
                st2 = pst.tile([128, 2, 512], F32, tag="ps", name="st")
                for j in range(2):
                    nc.tensor.matmul(
                        st2[:, j, :],
                        lhsT=kT[a:a + 32, hc, m * 128:(m + 1) * 128],
                        rhs=qT[a:a + 32, hc, j * 512:(j + 1) * 512],
                        start=True,
                        stop=True,
                        tile_position=(a, 0),
                    )
                return st2

            carry = []
            for ip, (hA, hB) in enumerate(PAIRS):
                pas = (
                    pap.tile([128, 264], F32, tag="pa", name=f"paA{ip}"),
                    pap.tile([128, 264], F32, tag="pa", name=f"paB{ip}"),
                )
                heads = (hA, hB)
                pend = []
                for m in range(8):
                    ph = {
                        hs: emit_exp(
                            "A" if hs == 0 else "V", emit_s_full(h, m))
                        for hs, h in ((0, hA), (1, hB))
                    }
                    pend.append((m, ph, None))
                    # carried PVs wait until m>=2 so the previous pair's
                    # trailing exps (still draining on DVE) don't head-of-line
                    # stall the PE queue
                    if carry and m >= 2:
                        carry.pop(0)()
                    pair_extra(ip, m)
                    # the last pair drains its PVs earlier to shorten the tail
                    if len(pend) > (1 if ip == 3 else 2):
                        e = pend.pop(0)
                        emit_pv(e[0], e[1], pas, heads, e[2])
                # defer the tail PVs + normalization into the next pair's
                # m-loop so the PE never waits on the trailing exps
                thunks = [
                    (lambda e=e, pas=pas, heads=heads: emit_pv(
                        e[0], e[1], pas, heads, e[2]))
                    for e in pend
                ]
                for hs in (0, 1):
                    thunks.append(
                        lambda hs=hs, pas=pas, heads=heads: emit_norm(
                            (pas[hs],), (heads[hs],))
                    )
                carry = thunks

            # ---- tail: last pair's PVs + norms first (they gate the whole
            # output chain), then the remaining chunk-1 projection ----
            for t in carry:  # PV(7) j-halves, the two norms
                t()
            emit_proj1_pair(3)

            if debug_dump:
                nc.sync.dma_start(dbg["d_yT"], yT.bitcast(F32))
                nc.sync.dma_start(dbg["d_qT"], qT.bitcast(F32))
                nc.sync.dma_start(dbg["d_kT"], kT.bitcast(F32))
                dvf = big.tile([128, 8, 264], F32, tag="dvf")
                nc.vector.tensor_copy(dvf, vsb)
                nc.sync.dma_start(dbg["d_v"], dvf)
                daf = big.tile([128, 8, 256], F32, tag="daf")
                nc.vector.tensor_copy(daf, a_sb)
                nc.sync.dma_start(dbg["d_asb"], daf)

            # transpose chunk-0 (shared-bank, half-copies so the first
            # projections start before the second half lands), project in
            # token-chunk pairs, re-add staged half via identity matmul,
            # merged copies, store
            tp0 = pst.tile([128, 1024], BF16, tag="ps", name="atp0")
            for i in range(4):
                emit_atr_mm(0, i, tp0)
            for i in range(4, 8):
                emit_atr_mm(0, i, tp0)
            nc.vector.tensor_copy(attnT[:, 0, 0:512], tp0[:, 0:512])
            nc.vector.tensor_copy(attnT[:, 0, 512:1024], tp0[:, 512:1024])
            for np_ in range(4):
                ops = pst.tile([128, 2, 512], F32, tag="ps", name="ops")
                for q in range(2):
                    nt = np_ * 2 + q
                    dst = ops[:, 0, q * 256:(q + 1) * 256]
                    nc.tensor.matmul(
                        dst,
                        lhsT=attnT[:, 0, nt * 128:(nt + 1) * 128],
                        rhs=outwT_sb[:, 0, :],
                        start=(q == 0),
                        stop=False,
                    )
                    nc.tensor.matmul(
                        dst,
                        lhsT=idr_sb,
                        rhs=partial1[:, nt, :],
                        start=False,
                        stop=(q == 1),
                    )
                osb2 = outs_p.tile([128, 2, C], F32, tag="o", name="osb2")
                # alternate engines: DVE is idle once the last norms are done
                copy_alt(osb2.rearrange("p a b -> p (a b)"), ops[:, 0, :])
                # one batched DMA per 2 token chunks (HWDGE overhead is
                # per-descriptor-set, ~625ns each)
                oq = nc.sync if np_ % 2 else nc.scalar
                oq.dma_start(
                    out_d[np_ * 256:(np_ + 1) * 256, :].rearrange(
                        "(c p) f -> p c f", p=128),
                    osb2,
                )

    nc.compile()
    return nc


_NC = None
LAST_RESULTS = None


def _host_prep(conv_w, conv_b, qkv_w, out_w, out_b):
    import ml_dtypes

    conv_w = np.asarray(conv_w, np.float32).reshape(C, 3, 3)
    w18 = np.zeros((128, 18), np.float32)
    for ct in range(2):
        for t, (ky, kx) in enumerate(TAPS):
            d = conv_w[128 * ct: 128 * (ct + 1), ky, kx].copy()
            if (ky, kx) == (1, 1):
                d += 1.0  # residual connection folded into the center tap
            w18[:, ct * 9 + t] = d
    blobA = np.zeros((128, BAW), ml_dtypes.bfloat16)
    blobA[:, BA_ID:BA_ID + 128] = np.eye(128, dtype=ml_dtypes.bfloat16)
    blobA[:, BA_W18:BA_W18 + 18] = w18.astype(ml_dtypes.bfloat16)
    cb = np.asarray(conv_b, np.float32).reshape(2, 128).T
    blobA[:, BA_CONVB:BA_CONVB + 2] = cb.astype(ml_dtypes.bfloat16)
    blobB = np.zeros((128, BBW), ml_dtypes.bfloat16)
    owT = np.ascontiguousarray(np.asarray(out_w, np.float32).T).astype(
        ml_dtypes.bfloat16)  # [256 in, 256 outc]
    blobB[:, BB_OWT:BB_OWT + 512] = np.concatenate(
        [owT[0:128, :], owT[128:256, :]], axis=1)
    blobB[0, BB_OUTB:BB_OUTB + 256] = np.asarray(out_b, np.float32).astype(
        ml_dtypes.bfloat16)
    return {
        "qkv_wT": np.ascontiguousarray(np.asarray(qkv_w, np.float32).T),
        "blobA": blobA,
        "blobB": blobB,
    }


def _prep_x(x):
    """bf16, host-transposed to [B, C, N] for straight (transpose-free) DMA."""
    import ml_dtypes

    xt = np.swapaxes(np.asarray(x, np.float32), -1, -2)
    return np.ascontiguousarray(xt.astype(ml_dtypes.bfloat16))


def kernel(x, conv_w, conv_b, qkv_w, out_w, out_b):
    global _NC, LAST_RESULTS

    if _NC is None:
        _NC = build_nc()
    x = _prep_x(x)
    shared = _host_prep(conv_w, conv_b, qkv_w, out_w, out_b)
    in_maps = [{**shared, "x": np.ascontiguousarray(x[b])} for b in range(B)]
    trace = bool(int(os.environ.get("KERNEL_TRACE", "0")))
    try:
        res = run_bass_kernel_spmd(_NC, in_maps, core_ids=list(range(B)), trace=trace)
    except Exception:
        if not trace:
            raise
        res = run_bass_kernel_spmd(_NC, in_maps, core_ids=list(range(B)), trace=False)
    LAST_RESULTS = res
    return np.stack([res.results[b]["out"] for b in range(B)], axis=0)


# revision 80
# speedup vs baseline: 1.0223x; 1.0105x over previous
"""Trainium2 Bass kernel for nn_Attention_43190191129190.

Model (per batch element b of 8):
    y   = x + dwconv3x3(x) + conv_b          (depthwise residual positional conv)
    qkv = y @ qkv_w.T ; split into q, k, v   (8 heads, dim 32)
    out = softmax(q k^T / sqrt(32)) v
    out = out @ out_w.T + out_b
Sharding: pure data-parallel, one batch element per NeuronCore (8 cores).

Per-core design (v5 — merged 2-bank tiles, wide exp, merged evacuations):

  The ACT+DVE engines are the wall: every S element must be exp'd
  (65536 cols of [128]-partition work) and every PSUM result must be
  evacuated by ACT/DVE (DMA and GPSIMD cannot touch PSUM). v5 cuts the
  per-instruction init overhead (ACT ~185ns, DVE ~125ns busy per op) by
  merging work into the widest possible instructions:

  1. x arrives bf16, host pre-transposed; 2 DMA-xbar transposes stage
     x^T, copied into a zero-haloed [C, 34, 34] image.
  2. diag conv matrices built on device by TWO [128,9,128]
     affine_selects (one per channel tile) instead of 18 narrow ones.
  3. conv per ct: one [128,2,512] PSUM tile, j halves as two 10-matmul
     accumulation groups, ONE [128,1024] evacuation (bias via K=1 tap).
  4. q^T/k^T per feature tile: one [128,2,512] tile, 4 matmuls, ONE
     evacuation. v: two 4-token-chunk units, 8 matmuls + ONE strided
     evacuation each into [v_h|1] 33-wide head slots (ones preset).
  5. Attention, head pair per generation, 8 m-steps each:
       S^T per (head, m): one [128,2,512] f32 PSUM tile (two 512-wide
       matmuls), then ONE 1024-wide exp:
         hs0 -> ACT exact Exp -> bf16; hs1 -> DVE Schraudolph
         (tensor_scalar s*A+B -> int16 bits == bf16(exp(s*SCALE))).
       PV unchanged: per-head [128, 8x33] PSUM accumulator, stationary
       p^T chunks, moving [v_h|1]; column 32 = softmax denominators;
       one accumulation group per bank. Norm per head: reciprocal +
       one broadcast tensor_tensor -> a_sb bf16.
  6. a_sb -> attnT: 8 transposes per ct share ONE [128,1024] bf16 bank
     (single accumulation group, disjoint regions), ONE 2x-mode DVE
     copy per ct.
  7. projection: chunk-1 + out_b staged mid-kernel into partial1 (pairs
     of token chunks share a bank, one [128,512] copy); tail re-adds
     partial1 via f32r identity matmuls and stores via merged copies.

  PSUM: 3x[128,2,512] f32 rotating slots + 2x[128,264] PV accumulators.
  Pre-attention work interleaves into the pair loops one self-contained
  slice per m-step (alloc+use+evacuate within the slice).
"""

import os

import numpy as np

import concourse.bass as bass
import concourse.tile as tile
from concourse import bacc, mybir
from concourse.bass_utils import run_bass_kernel_spmd

F32 = mybir.dt.float32
F32R = mybir.dt.float32r
BF16 = mybir.dt.bfloat16
I16 = mybir.dt.int16
AF = mybir.ActivationFunctionType
ALU = mybir.AluOpType

B, N, C = 8, 1024, 256
HEADS, DH = 8, 32
SCALE = DH ** -0.5
PAD = 34  # 32x32 spatial grid with 1-px halo

# blobA (bf16): id [128, 0:128] | w18 [128, 128:146] | convb cols [128, 146:148]
BA_ID, BA_W18, BA_CONVB, BAW = 0, 128, 146, 148
# blobB (bf16): outwT [128, 0:512] | outb row0 [512:768]
BB_OWT, BB_OUTB, BBW = 0, 512, 768

TAPS = [(ky, kx) for ky in range(3) for kx in range(3)]
# chunk-1 head pairs first so the chunk-1 projection can run mid-kernel;
# the tail then only waits on the last pair's (chunk-0) normalization
PAIRS = [(5, 7), (4, 6), (1, 3), (0, 2)]

# Schraudolph fast-exp: int16 bits of bf16(exp(s*SCALE)) = s*A + B
SCHR_C = 450000.0
SCHR_A = float(SCALE * (2 ** 23) / np.log(2) / 65536.0)
SCHR_B = float((127 * 2 ** 23 - SCHR_C) / 65536.0)


def build_nc(debug_dump=False):
    nc = bacc.Bacc("TRN2", target_bir_lowering=False, debug=False, num_devices=8)

    # x arrives host-pre-transposed: [C, N] bf16, one straight DMA
    x_d = nc.dram_tensor("x", (C, N), BF16, kind="ExternalInput").ap()
    qkvwT_d = nc.dram_tensor("qkv_wT", (C, 3 * C), F32R, kind="ExternalInput").ap()
    blobA_d = nc.dram_tensor("blobA", (128, BAW), BF16, kind="ExternalInput").ap()
    blobB_d = nc.dram_tensor("blobB", (128, BBW), BF16, kind="ExternalInput").ap()
    out_d = nc.dram_tensor("out", (N, C), F32, kind="ExternalOutput").ap()
    dbg = {}
    if debug_dump:
        for name, shape in (
            ("d_yT", (128, 2, N)), ("d_qT", (128, 2, N)), ("d_kT", (128, 2, N)),
            ("d_v", (128, 8, 264)), ("d_asb", (128, 8, 256)),
        ):
            dbg[name] = nc.dram_tensor(name, shape, F32, kind="ExternalOutput").ap()

    with tile.TileContext(nc) as tc:
        with (
            tc.tile_pool(name="const", bufs=1) as const,
            tc.tile_pool(name="big", bufs=1) as big,
            tc.tile_pool(name="pT", bufs=16) as ppool,
            tc.tile_pool(name="rcp", bufs=4) as rcp_p,
            tc.tile_pool(name="outs", bufs=4) as outs_p,
            tc.tile_pool(name="pst", bufs=3, space="PSUM") as pst,
            tc.tile_pool(name="pap", bufs=2, space="PSUM") as pap,
        ):
            # ---- persistent activations (x image first: DMA critical path)
            xpadT = big.tile([128, 2, PAD * PAD], BF16, tag="xpadT")
            xpv = xpadT.bitcast(mybir.dt.uint16).rearrange(
                "p ct (h w) -> p ct h w", h=PAD
            )
            nc.vector.memset(xpv[:, :, 0, :], 0)
            nc.vector.memset(xpv[:, :, PAD - 1, :], 0)
            nc.vector.memset(xpv[:, :, :, 0], 0)
            nc.vector.memset(xpv[:, :, :, PAD - 1], 0)

            # ---- DMAs. Per-DMA cost in the serial DMA pipeline is large
            # (HWDGE 625 + DGE delay 650 + transfer + completion sem 900),
            # so x is host-pre-transposed and lands in ONE straight DMA.
            blobA_sb = const.tile([128, BAW], BF16, tag="blobA")
            nc.sync.dma_start(blobA_sb, blobA_d)
            xstg = big.tile([128, 2, N], BF16, tag="xstg")
            for ct in range(2):
                nc.sync.dma_start(xstg[:, ct, :],
                                  x_d[ct * 128:(ct + 1) * 128, :])
            id_sb = blobA_sb[:, BA_ID:BA_ID + 128]
            w18_sb = blobA_sb[:, BA_W18:BA_W18 + 18]
            convb2_sb = blobA_sb[:, BA_CONVB:BA_CONVB + 2]
            qkvwT_sb = const.tile([128, 2, 3 * C], F32R, tag="qkvwT")
            for ct in range(2):
                nc.sync.dma_start(
                    qkvwT_sb[:, ct, 0:512],
                    qkvwT_d[ct * 128:(ct + 1) * 128, 0:512],
                )
            nc.sync.dma_start(
                qkvwT_sb[:, :, 512:768],
                qkvwT_d[:, 512:768].rearrange("(kc p) f -> p kc f", p=128),
            )
            blobB_sb = const.tile([128, BBW], BF16, tag="blobB")
            nc.sync.dma_start(blobB_sb, blobB_d)
            outwT_sb = blobB_sb[:, BB_OWT:BB_OWT + 512].rearrange(
                "p (kc f) -> p kc f", kc=2)
            outb_sb = blobB_sb[0:1, BB_OUTB:BB_OUTB + 256]

            # diag conv matrices: diag[c, t, f] = w18[c, t] * id[c, f] via
            # one DVE tensor_tensor per channel tile (DVE is idle at startup
            # and this beats the Pool affine_select by ~3us of latency)
            # ---- warm-ups (after the DMA issues so they don't block the
            # ACT queue): the exp ACT-table load and a chained trickle of
            # tiny PE matmuls (keeps the PE "recently active" through the
            # DMA wait so the conv burst is not dispatched into the cost
            # model's cold p-state)
            zerob_sb = const.tile([128, 1], F32, tag="zerob")
            nc.vector.memset(zerob_sb, 0.0)
            warm_sb = const.tile([1, 1], F32, tag="warm")
            nc.scalar.activation(
                warm_sb, zerob_sb[0:1, 0:1], AF.Exp,
                bias=zerob_sb[0:1], scale=1.0,
            )
            wv = const.tile([1, 20], F32, tag="wv")
            nc.vector.memset(wv, 0.0)
            for k in range(17):
                wps = pst.tile([128, 2, 512], F32, tag="ps", name="wps")
                nc.tensor.matmul(
                    wps[0:1, 0, 0:1], lhsT=wv[0:1, k:k + 1],
                    rhs=wv[0:1, k:k + 1], start=True, stop=True,
                )
                if k + 1 < 20:
                    nc.scalar.copy(wv[0:1, k + 1:k + 2], wps[0:1, 0, 0:1])

            diag_sb = const.tile([128, 18, 128], BF16, tag="diag")

            def emit_diag(ct):
                idb = bass.AP(
                    tensor=id_sb.tensor, offset=id_sb.offset,
                    ap=[list(id_sb.ap[0]), [0, 9], [1, 128]],
                )
                w18b = bass.AP(
                    tensor=w18_sb.tensor,
                    offset=w18_sb.offset + ct * 9,
                    ap=[list(w18_sb.ap[0]), [1, 9], [0, 128]],
                )
                nc.vector.tensor_tensor(
                    out=diag_sb[:, ct * 9:(ct + 1) * 9, :],
                    in0=idb, in1=w18b, op=ALU.mult,
                )

            def emit_xpad(ct):
                nc.vector.tensor_copy(
                    xpadT[:, ct, :].rearrange("p (h w) -> p h w", h=PAD)[
                        :, 1:33, 1:33
                    ],
                    xstg[:, ct, :].rearrange("p (h w) -> p h w", h=32),
                )

            # ones row generated on device (proj-bias rhs)
            ones_sb = const.tile([1, 512], BF16, tag="ones")
            nc.gpsimd.memset(ones_sb, 1.0)
            # conv bias in f32 for the per-partition bias of the conv
            # evacuation (folds the bias add into the PSUM->SBUF copy)
            convbf = const.tile([128, 2], F32, tag="convbf")
            # DVE order matters: ct0's conv inputs complete before ct1's
            # begin, so the ct0 conv matmuls start ~2us sooner
            nc.vector.tensor_copy(convbf, convb2_sb)
            emit_diag(0)
            emit_xpad(0)
            emit_diag(1)
            emit_xpad(1)

            yT = big.tile([128, 2, N], F32R, tag="yT")
            qT = big.tile([128, 2, N], F32R, tag="qT")
            kT = big.tile([128, 2, N], F32R, tag="kT")
            # [v_h | 1] per (token-chunk, head); ones preset via memset
            vsb = big.tile([128, 8, 8 * 33], BF16, tag="v")
            nc.gpsimd.memset(vsb, 1.0)
            a_sb = big.tile([128, 8, 256], BF16, tag="a_sb")
            attnT = big.tile([128, 2, N], BF16, tag="attnT")

            # psum evacuations: GPSIMD cannot access PSUM on HW, so they
            # alternate between the ACT (scalar.copy) and DVE engines
            _cp = [0]

            def copy_alt(dst, src_ap):
                _cp[0] += 1
                if _cp[0] % 2:
                    nc.scalar.copy(dst, src_ap)
                else:
                    nc.vector.tensor_copy(dst, src_ap)

            # ---- conv: per (ct, j) half: 9 diagonal matmuls + K=1 bias tap,
            # one 512-wide evacuation (j-split so the attention wavefront can
            # start on the j0 token half while j1 is still convolving)
            def emit_conv_half(ct, j):
                cps = pst.tile([128, 512], F32, tag="ps", name=f"cacc{ct}{j}")
                view = xpadT[:, ct, :].rearrange("p (h w) -> p h w", h=PAD)
                for t, (ky, kx) in enumerate(TAPS):
                    nc.tensor.matmul(
                        cps,
                        lhsT=diag_sb[:, ct * 9 + t, :],
                        rhs=view[:, ky + 16 * j: ky + 16 * j + 16, kx: kx + 32],
                        start=(t == 0),
                        stop=(t == 8),
                    )
                # conv bias folded into the evacuation (per-partition add)
                dst = yT[:, ct, j * 512:(j + 1) * 512]
                _cp[0] += 1
                if _cp[0] % 2:
                    nc.scalar.activation(
                        dst, cps, AF.Identity,
                        bias=convbf[:, ct:ct + 1], scale=1.0)
                else:
                    cb = bass.AP(
                        tensor=convbf.tensor, offset=convbf.offset + ct,
                        ap=[list(convbf.ap[0]), [0, 512]],
                    )
                    nc.vector.tensor_tensor(
                        out=dst, in0=cps, in1=cb, op=ALU.add)

            # ---- q^T / k^T: full feature tile or single token-half ----
            def emit_qk_half(ft, j):
                dstT, dc = (qT, ft) if ft < 2 else (kT, ft - 2)
                fofs = 0 if ft < 2 else 256
                qps = pst.tile([128, 512], F32, tag="ps", name="qps")
                for kc in range(2):
                    nc.tensor.matmul(
                        qps,
                        lhsT=qkvwT_sb[:, kc, fofs + dc * 128: fofs + (dc + 1) * 128],
                        rhs=yT[:, kc, j * 512:(j + 1) * 512],
                        start=(kc == 0),
                        stop=(kc == 1),
                    )
                copy_alt(dstT[:, dc, j * 512:(j + 1) * 512], qps)

            def emit_qk(ft):
                dstT, dc = (qT, ft) if ft < 2 else (kT, ft - 2)
                fofs = 0 if ft < 2 else 256
                qps = pst.tile([128, 2, 512], F32, tag="ps", name="qps")
                for j in range(2):
                    for kc in range(2):
                        nc.tensor.matmul(
                            qps[:, j, :],
                            lhsT=qkvwT_sb[:, kc, fofs + dc * 128: fofs + (dc + 1) * 128],
                            rhs=yT[:, kc, j * 512:(j + 1) * 512],
                            start=(kc == 0),
                            stop=(kc == 1),
                        )
                copy_alt(dstT[:, dc, :], qps.rearrange("p a b -> p (a b)"))

            # ---- v: 4 token chunks per unit, 8 matmuls, one strided evac ----
            def emit_v4(u):
                vps = pst.tile([128, 2, 512], F32, tag="ps", name="vps")
                for q in range(4):
                    nt = u * 4 + q
                    dst = vps[:, q // 2, (q % 2) * 256:(q % 2) * 256 + 256]
                    for kc in range(2):
                        # one open accumulation group per bank: start on the
                        # bank's first write, stop on its last
                        nc.tensor.matmul(
                            dst,
                            lhsT=yT[:, kc, nt * 128:(nt + 1) * 128],
                            rhs=qkvwT_sb[:, kc, 512:768],
                            start=(kc == 0 and q % 2 == 0),
                            stop=(kc == 1 and q % 2 == 1),
                        )
                sv = vps.rearrange("p a (q hh c) -> p (a q) hh c", q=2, c=32)
                dv = vsb[:, u * 4:(u + 1) * 4, :].rearrange(
                    "p n (hh c) -> p n hh c", c=33)[:, :, :, 0:32]
                copy_alt(dv, sv)

            # pre-loop: exactly what pair 0 needs up front (chunk-1 q/k and
            # the first four v chunks); the rest trickles in as one light
            # half-unit extra per m-step so the S/exp PSUM rotation is never
            # starved for long
            for ct in range(2):
                for j in range(2):
                    emit_conv_half(ct, j)
            emit_qk(1)
            emit_qk(3)
            emit_v4(0)

            # ---- a_sb -> attnT: 8 transposes sharing one bank + ONE copy ----
            def emit_atr_mm(ct, nc_i, tp):
                nc.tensor.matmul(
                    tp[:, nc_i * 128:(nc_i + 1) * 128],
                    lhsT=a_sb[:, nc_i, ct * 128:(ct + 1) * 128],
                    rhs=id_sb,
                    is_transpose=True,
                    start=(nc_i == 0),
                    stop=(nc_i == 7),
                )

            # interleaved extras, one self-contained slice per m-step
            def emit_atr_ct(ct):
                # all 8 transposes share one bank-tile + ONE 2x-mode copy;
                # single slice keeps the PSUM slot hold under ~1 m-step
                tp = pst.tile([128, 1024], BF16, tag="ps", name=f"atp{ct}")
                for i in range(8):
                    emit_atr_mm(ct, i, tp)
                nc.vector.tensor_copy(attnT[:, ct, :], tp)

            def pair_extra(ip, m):
                if ip == 0:
                    if m == 1:
                        emit_v4(1)
                    elif m == 3:
                        emit_qk_half(0, 0)
                    elif m == 5:
                        emit_qk_half(0, 1)
                elif ip == 1:
                    if m == 1:
                        emit_qk_half(2, 0)
                    elif m == 3:
                        emit_qk_half(2, 1)
                elif ip == 2:
                    if m == 6:
                        emit_atr_ct(1)

            # ---- merged exp: ONE 1024-wide instruction per (head, m).
            # hs0 sits on the 1-step-slack PSUM slot: its exp gates the
            # S-issue chain, so it always runs on the faster ACT engine.
            # hs1 (2-step slack) goes to DVE except two steps per pair,
            # balancing total engine busy (~42 ACT / 22 DVE tiles).
            def emit_exp_half(eng, sv, w):
                if eng == "A":
                    p = ppool.tile([128, w], BF16, tag="pT", name="pA")
                    nc.scalar.activation(p, sv, AF.Exp, bias=zerob_sb, scale=SCALE)
                    return p
                p = ppool.tile([128, w], I16, tag="pT", name="pV")
                nc.vector.tensor_scalar(
                    out=p, in0=sv, scalar1=SCHR_A, scalar2=SCHR_B,
                    op0=ALU.mult, op1=ALU.add,
                )
                return p.bitcast(BF16)

            def emit_exp(eng, st2):
                return emit_exp_half(
                    eng, st2.rearrange("p a b -> p (a b)"), 1024)

            # ---- attention ----
            def emit_pv(m, ph, pas, heads, rng=None):
                # one accumulation group per pa bank: start only on the first
                # write (lazy 2KB region-zeroing covers the other 7
                # sub-regions), stop only on the last. rng selects a 4-chunk
                # n-range for the pair-0 wavefront half-tiles.
                base = 0 if rng is None else rng
                for nc_i in (range(8) if rng is None else range(rng, rng + 4)):
                    for hs in (0, 1):
                        nc.tensor.matmul(
                            pas[hs][:, nc_i * 33: nc_i * 33 + 33],
                            lhsT=ph[hs][:, (nc_i - base) * 128:
                                        (nc_i - base + 1) * 128],
                            rhs=vsb[:, m, 33 * heads[hs]: 33 * heads[hs] + 33],
                            start=(m == 0 and nc_i == 0),
                            stop=(m == 7 and nc_i == 7),
                        )

            def emit_norm(pas, heads):
                for h, pa in zip(heads, pas):
                    pav = pa.rearrange("p (nc e) -> p nc e", e=33)
                    rcp = rcp_p.tile([128, 8], F32, tag="rcp", name="rcp")
                    nc.vector.reciprocal(rcp, pav[:, :, 32])
                    rcp_b = bass.AP(
                        tensor=rcp.tensor, offset=rcp.offset,
                        ap=[list(rcp.ap[0]), [1, 8], [0, 32]],
                    )
                    nc.vector.tensor_tensor(
                        out=a_sb[:, :, h * 32: h * 32 + 32],
                        in0=pav[:, :, 0:32],
                        in1=rcp_b,
                        op=ALU.mult,
                    )

            def emit_s_half(h, m, j):
                a = 32 * (h % 4)
                hc = h // 4
                sth = pst.tile([128, 512], F32, tag="ps", name="sth")
                nc.tensor.matmul(
                    sth,
                    lhsT=kT[a:a + 32, hc, m * 128:(m + 1) * 128],
                    rhs=qT[a:a + 32, hc, j * 512:(j + 1) * 512],
                    start=True,
                    stop=True,
                    tile_position=(a, 0),
                )
                return sth

            def emit_s_full(h, m):
                a = 32 * (h % 4)
                hc = h // 4
                st2 = pst.tile([128, 2, 512], F32, tag="ps", name="st")
                for j in range(2):
                    nc.tensor.matmul(
                        st2[:, j, :],
                        lhsT=kT[a:a + 32, hc, m * 128:(m + 1) * 128],
                        rhs=qT[a:a + 32, hc, j * 512:(j + 1) * 512],
                        start=True,
                        stop=True,
                        tile_position=(a, 0),
                    )
                return st2

            carry = []
            for ip, (hA, hB) in enumerate(PAIRS):
                pas = (
                    pap.tile([128, 264], F32, tag="pa", name=f"paA{ip}"),
                    pap.tile([128, 264], F32, tag="pa", name=f"paB{ip}"),
                )
                heads = (hA, hB)
                pend = []
                for m in range(8):
                    ph = {
                        hs: emit_exp(
                            "A" if hs == 0 else "V", emit_s_full(h, m))
                        for hs, h in ((0, hA), (1, hB))
                    }
                    pend.append((m, ph, None))
                    # carried PVs wait until m>=2 so the previous pair's
                    # trailing exps (still draining on DVE) don't head-of-line
                    # stall the PE queue
                    if carry and m >= 2:
                        carry.pop(0)()
                    pair_extra(ip, m)
                    # the last pair drains its PVs earlier to shorten the tail
                    if len(pend) > (1 if ip == 3 else 2):
                        e = pend.pop(0)
                        emit_pv(e[0], e[1], pas, heads, e[2])
                # defer the tail PVs + normalization into the next pair's
                # m-loop so the PE never waits on the trailing exps
                thunks = [
                    (lambda e=e, pas=pas, heads=heads: emit_pv(
                        e[0], e[1], pas, heads, e[2]))
                    for e in pend
                ]
                for hs in (0, 1):
                    thunks.append(
                        lambda hs=hs, pas=pas, heads=heads: emit_norm(
                            (pas[hs],), (heads[hs],))
                    )
                carry = thunks

            # ---- tail: last pair's PVs + norms first (they gate the whole
            # output chain) ----
            for t in carry:  # PV(7), the two norms
                t()

            if debug_dump:
                nc.sync.dma_start(dbg["d_yT"], yT.bitcast(F32))
                nc.sync.dma_start(dbg["d_qT"], qT.bitcast(F32))
                nc.sync.dma_start(dbg["d_kT"], kT.bitcast(F32))
                dvf = big.tile([128, 8, 264], F32, tag="dvf")
                nc.vector.tensor_copy(dvf, vsb)
                nc.sync.dma_start(dbg["d_v"], dvf)
                daf = big.tile([128, 8, 256], F32, tag="daf")
                nc.vector.tensor_copy(daf, a_sb)
                nc.sync.dma_start(dbg["d_asb"], daf)

            # transpose chunk-0 (shared-bank, half-copies so the first
            # projections start before the second half lands), project in
            # token-chunk pairs, re-add staged half via identity matmul,
            # merged copies, store
            tp0 = pst.tile([128, 1024], BF16, tag="ps", name="atp0")
            for i in range(4):
                emit_atr_mm(0, i, tp0)
            for i in range(4, 8):
                emit_atr_mm(0, i, tp0)
            nc.vector.tensor_copy(attnT[:, 0, 0:512], tp0[:, 0:512])
            nc.vector.tensor_copy(attnT[:, 0, 512:1024], tp0[:, 512:1024])
            for np_ in range(4):
                ops = pst.tile([128, 2, 512], F32, tag="ps", name="ops")
                for q in range(2):
                    nt = np_ * 2 + q
                    dst = ops[:, 0, q * 256:(q + 1) * 256]
                    for kc in range(2):
                        nc.tensor.matmul(
                            dst,
                            lhsT=attnT[:, kc, nt * 128:(nt + 1) * 128],
                            rhs=outwT_sb[:, kc, :],
                            start=(q == 0 and kc == 0),
                            stop=False,
                        )
                # out_b as a K=1 tap over the whole pair bank
                ob = bass.AP(
                    tensor=outb_sb.tensor, offset=outb_sb.offset,
                    ap=[list(outb_sb.ap[0]), [0, 2], [1, 256]],
                )
                nc.tensor.matmul(
                    ops[:, 0, :],
                    lhsT=ones_sb[0:1, 0:128],
                    rhs=ob,
                    start=False,
                    stop=True,
                )
                osb2 = outs_p.tile([128, 2, C], F32, tag="o", name="osb2")
                # alternate engines: DVE is idle once the last norms are done
                copy_alt(osb2.rearrange("p a b -> p (a b)"), ops[:, 0, :])
                # one batched DMA per 2 token chunks (HWDGE overhead is
                # per-descriptor-set, ~625ns each)
                oq = nc.sync if np_ % 2 else nc.scalar
                oq.dma_start(
                    out_d[np_ * 256:(np_ + 1) * 256, :].rearrange(
                        "(c p) f -> p c f", p=128),
                    osb2,
                )

    nc.compile()
    return nc


_NC = None
LAST_RESULTS = None


def _host_prep(conv_w, conv_b, qkv_w, out_w, out_b):
    import ml_dtypes

    conv_w = np.asarray(conv_w, np.float32).reshape(C, 3, 3)
    w18 = np.zeros((128, 18), np.float32)
    for ct in range(2):
        for t, (ky, kx) in enumerate(TAPS):
            d = conv_w[128 * ct: 128 * (ct + 1), ky, kx].copy()
            if (ky, kx) == (1, 1):
                d += 1.0  # residual connection folded into the center tap
            w18[:, ct * 9 + t] = d
    blobA = np.zeros((128, BAW), ml_dtypes.bfloat16)
    blobA[:, BA_ID:BA_ID + 128] = np.eye(128, dtype=ml_dtypes.bfloat16)
    blobA[:, BA_W18:BA_W18 + 18] = w18.astype(ml_dtypes.bfloat16)
    cb = np.asarray(conv_b, np.float32).reshape(2, 128).T
    blobA[:, BA_CONVB:BA_CONVB + 2] = cb.astype(ml_dtypes.bfloat16)
    blobB = np.zeros((128, BBW), ml_dtypes.bfloat16)
    owT = np.ascontiguousarray(np.asarray(out_w, np.float32).T).astype(
        ml_dtypes.bfloat16)  # [256 in, 256 outc]
    blobB[:, BB_OWT:BB_OWT + 512] = np.concatenate(
        [owT[0:128, :], owT[128:256, :]], axis=1)
    blobB[0, BB_OUTB:BB_OUTB + 256] = np.asarray(out_b, np.float32).astype(
        ml_dtypes.bfloat16)
    return {
        "qkv_wT": np.ascontiguousarray(np.asarray(qkv_w, np.float32).T),
        "blobA": blobA,
        "blobB": blobB,
    }


def _prep_x(x):
    """bf16, host-transposed to [B, C, N] for straight (transpose-free) DMA."""
    import ml_dtypes

    xt = np.swapaxes(np.asarray(x, np.float32), -1, -2)
    return np.ascontiguousarray(xt.astype(ml_dtypes.bfloat16))


def kernel(x, conv_w, conv_b, qkv_w, out_w, out_b):
    global _NC, LAST_RESULTS

    if _NC is None:
        _NC = build_nc()
    x = _prep_x(x)
    shared = _host_prep(conv_w, conv_b, qkv_w, out_w, out_b)
    in_maps = [{**shared, "x": np.ascontiguousarray(x[b])} for b in range(B)]
    trace = bool(int(os.environ.get("KERNEL_TRACE", "0")))
    try:
        res = run_bass_kernel_spmd(_NC, in_maps, core_ids=list(range(B)), trace=trace)
    except Exception:
        if not trace:
            raise
        res = run_bass_kernel_spmd(_NC, in_maps, core_ids=list(range(B)), trace=False)
    LAST_RESULTS = res
    return np.stack([res.results[b]["out"] for b in range(B)], axis=0)


# revision 82
# speedup vs baseline: 1.0423x; 1.0195x over previous
"""Trainium2 Bass kernel for nn_Attention_43190191129190.

Model (per batch element b of 8):
    y   = x + dwconv3x3(x) + conv_b          (depthwise residual positional conv)
    qkv = y @ qkv_w.T ; split into q, k, v   (8 heads, dim 32)
    out = softmax(q k^T / sqrt(32)) v
    out = out @ out_w.T + out_b
Sharding: pure data-parallel, one batch element per NeuronCore (8 cores).

Per-core design (v5 — merged 2-bank tiles, wide exp, merged evacuations):

  The ACT+DVE engines are the wall: every S element must be exp'd
  (65536 cols of [128]-partition work) and every PSUM result must be
  evacuated by ACT/DVE (DMA and GPSIMD cannot touch PSUM). v5 cuts the
  per-instruction init overhead (ACT ~185ns, DVE ~125ns busy per op) by
  merging work into the widest possible instructions:

  1. x arrives bf16, host pre-transposed; 2 DMA-xbar transposes stage
     x^T, copied into a zero-haloed [C, 34, 34] image.
  2. diag conv matrices built on device by TWO [128,9,128]
     affine_selects (one per channel tile) instead of 18 narrow ones.
  3. conv per ct: one [128,2,512] PSUM tile, j halves as two 10-matmul
     accumulation groups, ONE [128,1024] evacuation (bias via K=1 tap).
  4. q^T/k^T per feature tile: one [128,2,512] tile, 4 matmuls, ONE
     evacuation. v: two 4-token-chunk units, 8 matmuls + ONE strided
     evacuation each into [v_h|1] 33-wide head slots (ones preset).
  5. Attention, head pair per generation, 8 m-steps each:
       S^T per (head, m): one [128,2,512] f32 PSUM tile (two 512-wide
       matmuls), then ONE 1024-wide exp:
         hs0 -> ACT exact Exp -> bf16; hs1 -> DVE Schraudolph
         (tensor_scalar s*A+B -> int16 bits == bf16(exp(s*SCALE))).
       PV unchanged: per-head [128, 8x33] PSUM accumulator, stationary
       p^T chunks, moving [v_h|1]; column 32 = softmax denominators;
       one accumulation group per bank. Norm per head: reciprocal +
       one broadcast tensor_tensor -> a_sb bf16.
  6. a_sb -> attnT: 8 transposes per ct share ONE [128,1024] bf16 bank
     (single accumulation group, disjoint regions), ONE 2x-mode DVE
     copy per ct.
  7. projection: chunk-1 + out_b staged mid-kernel into partial1 (pairs
     of token chunks share a bank, one [128,512] copy); tail re-adds
     partial1 via f32r identity matmuls and stores via merged copies.

  PSUM: 3x[128,2,512] f32 rotating slots + 2x[128,264] PV accumulators.
  Pre-attention work interleaves into the pair loops one self-contained
  slice per m-step (alloc+use+evacuate within the slice).
"""

import os

import numpy as np

import concourse.bass as bass
import concourse.tile as tile
from concourse import bacc, mybir
from concourse.bass_utils import run_bass_kernel_spmd

F32 = mybir.dt.float32
F32R = mybir.dt.float32r
BF16 = mybir.dt.bfloat16
I16 = mybir.dt.int16
AF = mybir.ActivationFunctionType
ALU = mybir.AluOpType

B, N, C = 8, 1024, 256
HEADS, DH = 8, 32
SCALE = DH ** -0.5
PAD = 34  # 32x32 spatial grid with 1-px halo

# blobA (bf16): id [128, 0:128] | w18 [128, 128:146] | convb cols [128, 146:148]
BA_ID, BA_W18, BA_CONVB, BAW = 0, 128, 146, 148
# blobB (bf16): outwT [128, 0:512] | outb row0 [512:768]
BB_OWT, BB_OUTB, BBW = 0, 512, 768

TAPS = [(ky, kx) for ky in range(3) for kx in range(3)]
# chunk-1 head pairs first so the chunk-1 projection can run mid-kernel;
# the tail then only waits on the last pair's (chunk-0) normalization
PAIRS = [(5, 7), (4, 6), (1, 3), (0, 2)]

# Schraudolph fast-exp: int16 bits of bf16(exp(s*SCALE)) = s*A + B
SCHR_C = 450000.0
SCHR_A = float(SCALE * (2 ** 23) / np.log(2) / 65536.0)
SCHR_B = float((127 * 2 ** 23 - SCHR_C) / 65536.0)


def build_nc(debug_dump=False):
    nc = bacc.Bacc("TRN2", target_bir_lowering=False, debug=False, num_devices=8)

    # x arrives host-pre-transposed: [C, N] bf16, one straight DMA
    x_d = nc.dram_tensor("x", (C, N), BF16, kind="ExternalInput").ap()
    qkvwT_d = nc.dram_tensor("qkv_wT", (C, 3 * C), F32R, kind="ExternalInput").ap()
    blobA_d = nc.dram_tensor("blobA", (128, BAW), BF16, kind="ExternalInput").ap()
    blobB_d = nc.dram_tensor("blobB", (128, BBW), BF16, kind="ExternalInput").ap()
    out_d = nc.dram_tensor("out", (N, C), F32, kind="ExternalOutput").ap()
    dbg = {}
    if debug_dump:
        for name, shape in (
            ("d_yT", (128, 2, N)), ("d_qT", (128, 2, N)), ("d_kT", (128, 2, N)),
            ("d_v", (128, 8, 264)), ("d_asb", (128, 8, 256)),
        ):
            dbg[name] = nc.dram_tensor(name, shape, F32, kind="ExternalOutput").ap()

    with tile.TileContext(nc) as tc:
        with (
            tc.tile_pool(name="const", bufs=1) as const,
            tc.tile_pool(name="big", bufs=1) as big,
            tc.tile_pool(name="pT", bufs=16) as ppool,
            tc.tile_pool(name="rcp", bufs=4) as rcp_p,
            tc.tile_pool(name="outs", bufs=4) as outs_p,
            tc.tile_pool(name="pst", bufs=3, space="PSUM") as pst,
            tc.tile_pool(name="pap", bufs=2, space="PSUM") as pap,
        ):
            # ---- persistent activations (x image first: DMA critical path)
            xpadT = big.tile([128, 2, PAD * PAD], BF16, tag="xpadT")
            xpv = xpadT.bitcast(mybir.dt.uint16).rearrange(
                "p ct (h w) -> p ct h w", h=PAD
            )
            nc.vector.memset(xpv[:, :, 0, :], 0)
            nc.vector.memset(xpv[:, :, PAD - 1, :], 0)
            nc.vector.memset(xpv[:, :, :, 0], 0)
            nc.vector.memset(xpv[:, :, :, PAD - 1], 0)

            # ---- DMAs. Per-DMA cost in the serial DMA pipeline is large
            # (HWDGE 625 + DGE delay 650 + transfer + completion sem 900),
            # so x is host-pre-transposed and lands in ONE straight DMA.
            blobA_sb = const.tile([128, BAW], BF16, tag="blobA")
            nc.sync.dma_start(blobA_sb, blobA_d)
            xstg = big.tile([128, 2, N], BF16, tag="xstg")
            for ct in range(2):
                nc.sync.dma_start(xstg[:, ct, :],
                                  x_d[ct * 128:(ct + 1) * 128, :])
            id_sb = blobA_sb[:, BA_ID:BA_ID + 128]
            w18_sb = blobA_sb[:, BA_W18:BA_W18 + 18]
            convb2_sb = blobA_sb[:, BA_CONVB:BA_CONVB + 2]
            qkvwT_sb = const.tile([128, 2, 3 * C], F32R, tag="qkvwT")
            for ct in range(2):
                nc.sync.dma_start(
                    qkvwT_sb[:, ct, 0:512],
                    qkvwT_d[ct * 128:(ct + 1) * 128, 0:512],
                )
            nc.sync.dma_start(
                qkvwT_sb[:, :, 512:768],
                qkvwT_d[:, 512:768].rearrange("(kc p) f -> p kc f", p=128),
            )
            blobB_sb = const.tile([128, BBW], BF16, tag="blobB")
            nc.sync.dma_start(blobB_sb, blobB_d)
            outwT_sb = blobB_sb[:, BB_OWT:BB_OWT + 512].rearrange(
                "p (kc f) -> p kc f", kc=2)
            outb_sb = blobB_sb[0:1, BB_OUTB:BB_OUTB + 256]

            # diag conv matrices: diag[c, t, f] = w18[c, t] * id[c, f] via
            # one DVE tensor_tensor per channel tile (DVE is idle at startup
            # and this beats the Pool affine_select by ~3us of latency)
            # ---- warm-ups (after the DMA issues so they don't block the
            # ACT queue): the exp ACT-table load and a chained trickle of
            # tiny PE matmuls (keeps the PE "recently active" through the
            # DMA wait so the conv burst is not dispatched into the cost
            # model's cold p-state)
            zerob_sb = const.tile([128, 1], F32, tag="zerob")
            nc.vector.memset(zerob_sb, 0.0)
            warm_sb = const.tile([1, 1], F32, tag="warm")
            nc.scalar.activation(
                warm_sb, zerob_sb[0:1, 0:1], AF.Exp,
                bias=zerob_sb[0:1], scale=1.0,
            )
            wv = const.tile([1, 20], F32, tag="wv")
            nc.vector.memset(wv, 0.0)
            for k in range(17):
                wps = pst.tile([128, 2, 512], F32, tag="ps", name="wps")
                nc.tensor.matmul(
                    wps[0:1, 0, 0:1], lhsT=wv[0:1, k:k + 1],
                    rhs=wv[0:1, k:k + 1], start=True, stop=True,
                )
                if k + 1 < 20:
                    nc.scalar.copy(wv[0:1, k + 1:k + 2], wps[0:1, 0, 0:1])

            diag_sb = const.tile([128, 18, 128], BF16, tag="diag")

            def emit_diag(ct):
                idb = bass.AP(
                    tensor=id_sb.tensor, offset=id_sb.offset,
                    ap=[list(id_sb.ap[0]), [0, 9], [1, 128]],
                )
                w18b = bass.AP(
                    tensor=w18_sb.tensor,
                    offset=w18_sb.offset + ct * 9,
                    ap=[list(w18_sb.ap[0]), [1, 9], [0, 128]],
                )
                nc.vector.tensor_tensor(
                    out=diag_sb[:, ct * 9:(ct + 1) * 9, :],
                    in0=idb, in1=w18b, op=ALU.mult,
                )

            def emit_xpad(ct):
                nc.vector.tensor_copy(
                    xpadT[:, ct, :].rearrange("p (h w) -> p h w", h=PAD)[
                        :, 1:33, 1:33
                    ],
                    xstg[:, ct, :].rearrange("p (h w) -> p h w", h=32),
                )

            # ones row generated on device (proj-bias rhs)
            ones_sb = const.tile([1, 512], BF16, tag="ones")
            nc.gpsimd.memset(ones_sb, 1.0)
            # conv bias in f32 for the per-partition bias of the conv
            # evacuation (folds the bias add into the PSUM->SBUF copy)
            convbf = const.tile([128, 2], F32, tag="convbf")
            # DVE order matters: ct0's conv inputs complete before ct1's
            # begin, so the ct0 conv matmuls start ~2us sooner
            nc.vector.tensor_copy(convbf, convb2_sb)
            emit_diag(0)
            emit_xpad(0)
            emit_diag(1)
            emit_xpad(1)

            yT = big.tile([128, 2, N], F32R, tag="yT")
            qT = big.tile([128, 2, N], F32R, tag="qT")
            kT = big.tile([128, 2, N], F32R, tag="kT")
            # [v_h | 1] per (token-chunk, head); ones preset via memset
            vsb = big.tile([128, 8, 8 * 33], BF16, tag="v")
            nc.gpsimd.memset(vsb, 1.0)
            a_sb = big.tile([128, 8, 256], BF16, tag="a_sb")
            attnT = big.tile([128, 2, N], BF16, tag="attnT")

            # psum evacuations: GPSIMD cannot access PSUM on HW, so they
            # alternate between the ACT (scalar.copy) and DVE engines
            _cp = [0]

            def copy_alt(dst, src_ap):
                _cp[0] += 1
                if _cp[0] % 2:
                    nc.scalar.copy(dst, src_ap)
                else:
                    nc.vector.tensor_copy(dst, src_ap)

            # ---- conv: per (ct, j) half: 9 diagonal matmuls + K=1 bias tap,
            # one 512-wide evacuation (j-split so the attention wavefront can
            # start on the j0 token half while j1 is still convolving)
            def emit_conv_half(ct, j):
                cps = pst.tile([128, 512], F32, tag="ps", name=f"cacc{ct}{j}")
                view = xpadT[:, ct, :].rearrange("p (h w) -> p h w", h=PAD)
                for t, (ky, kx) in enumerate(TAPS):
                    nc.tensor.matmul(
                        cps,
                        lhsT=diag_sb[:, ct * 9 + t, :],
                        rhs=view[:, ky + 16 * j: ky + 16 * j + 16, kx: kx + 32],
                        start=(t == 0),
                        stop=(t == 8),
                    )
                # conv bias folded into the evacuation (per-partition add)
                dst = yT[:, ct, j * 512:(j + 1) * 512]
                _cp[0] += 1
                if _cp[0] % 2:
                    nc.scalar.activation(
                        dst, cps, AF.Identity,
                        bias=convbf[:, ct:ct + 1], scale=1.0)
                else:
                    cb = bass.AP(
                        tensor=convbf.tensor, offset=convbf.offset + ct,
                        ap=[list(convbf.ap[0]), [0, 512]],
                    )
                    nc.vector.tensor_tensor(
                        out=dst, in0=cps, in1=cb, op=ALU.add)

            # ---- q^T / k^T: full feature tile or single token-half ----
            def emit_qk_half(ft, j):
                dstT, dc = (qT, ft) if ft < 2 else (kT, ft - 2)
                fofs = 0 if ft < 2 else 256
                qps = pst.tile([128, 512], F32, tag="ps", name="qps")
                for kc in range(2):
                    nc.tensor.matmul(
                        qps,
                        lhsT=qkvwT_sb[:, kc, fofs + dc * 128: fofs + (dc + 1) * 128],
                        rhs=yT[:, kc, j * 512:(j + 1) * 512],
                        start=(kc == 0),
                        stop=(kc == 1),
                    )
                copy_alt(dstT[:, dc, j * 512:(j + 1) * 512], qps)

            def emit_qk(ft):
                dstT, dc = (qT, ft) if ft < 2 else (kT, ft - 2)
                fofs = 0 if ft < 2 else 256
                qps = pst.tile([128, 2, 512], F32, tag="ps", name="qps")
                for j in range(2):
                    for kc in range(2):
                        nc.tensor.matmul(
                            qps[:, j, :],
                            lhsT=qkvwT_sb[:, kc, fofs + dc * 128: fofs + (dc + 1) * 128],
                            rhs=yT[:, kc, j * 512:(j + 1) * 512],
                            start=(kc == 0),
                            stop=(kc == 1),
                        )
                copy_alt(dstT[:, dc, :], qps.rearrange("p a b -> p (a b)"))

            # ---- v: 4 token chunks per unit, 8 matmuls, one strided evac ----
            def emit_v4(u):
                vps = pst.tile([128, 2, 512], F32, tag="ps", name="vps")
                for q in range(4):
                    nt = u * 4 + q
                    dst = vps[:, q // 2, (q % 2) * 256:(q % 2) * 256 + 256]
                    for kc in range(2):
                        # one open accumulation group per bank: start on the
                        # bank's first write, stop on its last
                        nc.tensor.matmul(
                            dst,
                            lhsT=yT[:, kc, nt * 128:(nt + 1) * 128],
                            rhs=qkvwT_sb[:, kc, 512:768],
                            start=(kc == 0 and q % 2 == 0),
                            stop=(kc == 1 and q % 2 == 1),
                        )
                sv = vps.rearrange("p a (q hh c) -> p (a q) hh c", q=2, c=32)
                dv = vsb[:, u * 4:(u + 1) * 4, :].rearrange(
                    "p n (hh c) -> p n hh c", c=33)[:, :, :, 0:32]
                copy_alt(dv, sv)

            # pre-loop: exactly what pair 0 needs up front (chunk-1 q/k and
            # the first four v chunks); the rest trickles in as one light
            # half-unit extra per m-step so the S/exp PSUM rotation is never
            # starved for long
            for ct in range(2):
                for j in range(2):
                    emit_conv_half(ct, j)
            emit_qk(1)
            emit_qk(3)
            emit_v4(0)

            # ---- a_sb -> attnT: 8 transposes sharing one bank + ONE copy ----
            def emit_atr_mm(ct, nc_i, tp):
                nc.tensor.matmul(
                    tp[:, nc_i * 128:(nc_i + 1) * 128],
                    lhsT=a_sb[:, nc_i, ct * 128:(ct + 1) * 128],
                    rhs=id_sb,
                    is_transpose=True,
                    start=(nc_i == 0),
                    stop=(nc_i == 7),
                )

            # interleaved extras, one self-contained slice per m-step
            def emit_atr_ct(ct):
                # all 8 transposes share one bank-tile + ONE 2x-mode copy;
                # single slice keeps the PSUM slot hold under ~1 m-step
                tp = pst.tile([128, 1024], BF16, tag="ps", name=f"atp{ct}")
                for i in range(8):
                    emit_atr_mm(ct, i, tp)
                nc.vector.tensor_copy(attnT[:, ct, :], tp)

            def pair_extra(ip, m):
                if ip == 0:
                    if m == 1:
                        emit_v4(1)
                    elif m == 3:
                        emit_qk_half(0, 0)
                    elif m == 5:
                        emit_qk_half(0, 1)
                elif ip == 1:
                    if m == 1:
                        emit_qk_half(2, 0)
                    elif m == 3:
                        emit_qk_half(2, 1)
                elif ip == 2:
                    if m == 6:
                        emit_atr_ct(1)

            # ---- merged exp: ONE 1024-wide instruction per (head, m).
            # hs0 sits on the 1-step-slack PSUM slot: its exp gates the
            # S-issue chain, so it always runs on the faster ACT engine.
            # hs1 (2-step slack) goes to DVE except two steps per pair,
            # balancing total engine busy (~42 ACT / 22 DVE tiles).
            def emit_exp_half(eng, sv, w):
                if eng == "A":
                    p = ppool.tile([128, w], BF16, tag="pT", name="pA")
                    nc.scalar.activation(p, sv, AF.Exp, bias=zerob_sb, scale=SCALE)
                    return p
                p = ppool.tile([128, w], I16, tag="pT", name="pV")
                nc.vector.tensor_scalar(
                    out=p, in0=sv, scalar1=SCHR_A, scalar2=SCHR_B,
                    op0=ALU.mult, op1=ALU.add,
                )
                return p.bitcast(BF16)

            def emit_exp(eng, st2):
                return emit_exp_half(
                    eng, st2.rearrange("p a b -> p (a b)"), 1024)

            # ---- attention ----
            def emit_pv(m, ph, pas, heads, rng=None):
                # one accumulation group per pa bank: start only on the first
                # write (lazy 2KB region-zeroing covers the other 7
                # sub-regions), stop only on the last. rng selects a 4-chunk
                # n-range for the pair-0 wavefront half-tiles.
                base = 0 if rng is None else rng
                for nc_i in (range(8) if rng is None else range(rng, rng + 4)):
                    for hs in (0, 1):
                        nc.tensor.matmul(
                            pas[hs][:, nc_i * 33: nc_i * 33 + 33],
                            lhsT=ph[hs][:, (nc_i - base) * 128:
                                        (nc_i - base + 1) * 128],
                            rhs=vsb[:, m, 33 * heads[hs]: 33 * heads[hs] + 33],
                            start=(m == 0 and nc_i == 0),
                            stop=(m == 7 and nc_i == 7),
                        )

            def emit_norm(pas, heads, via_pool=False):
                for h, pa in zip(heads, pas):
                    pav = pa.rearrange("p (nc e) -> p nc e", e=33)
                    rcp = rcp_p.tile([128, 8], F32, tag="rcp", name="rcp")
                    nc.vector.reciprocal(rcp, pav[:, :, 32])
                    rcp_b = bass.AP(
                        tensor=rcp.tensor, offset=rcp.offset,
                        ap=[list(rcp.ap[0]), [1, 8], [0, 32]],
                    )
                    if via_pool:
                        # DVE is the loop's ceiling engine: stage the PSUM
                        # accumulator to SBUF on ACT, then run the broadcast
                        # multiply on the otherwise-idle Pool engine
                        psb = rcp_p.tile([128, 264], F32, tag="nrm",
                                         name="psb")
                        nc.scalar.copy(psb, pa)
                        pv = psb.rearrange("p (nc e) -> p nc e", e=33)
                        nc.gpsimd.tensor_tensor(
                            out=a_sb[:, :, h * 32: h * 32 + 32],
                            in0=pv[:, :, 0:32],
                            in1=rcp_b,
                            op=ALU.mult,
                        )
                    else:
                        nc.vector.tensor_tensor(
                            out=a_sb[:, :, h * 32: h * 32 + 32],
                            in0=pav[:, :, 0:32],
                            in1=rcp_b,
                            op=ALU.mult,
                        )

            def emit_s_half(h, m, j):
                a = 32 * (h % 4)
                hc = h // 4
                sth = pst.tile([128, 512], F32, tag="ps", name="sth")
                nc.tensor.matmul(
                    sth,
                    lhsT=kT[a:a + 32, hc, m * 128:(m + 1) * 128],
                    rhs=qT[a:a + 32, hc, j * 512:(j + 1) * 512],
                    start=True,
                    stop=True,
                    tile_position=(a, 0),
                )
                return sth

            def emit_s_full(h, m):
                a = 32 * (h % 4)
                hc = h // 4
                st2 = pst.tile([128, 2, 512], F32, tag="ps", name="st")
                for j in range(2):
                    nc.tensor.matmul(
                        st2[:, j, :],
                        lhsT=kT[a:a + 32, hc, m * 128:(m + 1) * 128],
                        rhs=qT[a:a + 32, hc, j * 512:(j + 1) * 512],
                        start=True,
                        stop=True,
                        tile_position=(a, 0),
                    )
                return st2

            carry = []
            for ip, (hA, hB) in enumerate(PAIRS):
                pas = (
                    pap.tile([128, 264], F32, tag="pa", name=f"paA{ip}"),
                    pap.tile([128, 264], F32, tag="pa", name=f"paB{ip}"),
                )
                heads = (hA, hB)
                pend = []
                for m in range(8):
                    ph = {
                        hs: emit_exp(
                            "A" if hs == 0 else "V", emit_s_full(h, m))
                        for hs, h in ((0, hA), (1, hB))
                    }
                    pend.append((m, ph, None))
                    # carried PVs wait until m>=2 so the previous pair's
                    # trailing exps (still draining on DVE) don't head-of-line
                    # stall the PE queue
                    if carry and m >= 2:
                        carry.pop(0)()
                    pair_extra(ip, m)
                    # the last pair drains its PVs earlier to shorten the tail
                    if len(pend) > (1 if ip == 3 else 2):
                        e = pend.pop(0)
                        emit_pv(e[0], e[1], pas, heads, e[2])
                # defer the tail PVs + normalization into the next pair's
                # m-loop so the PE never waits on the trailing exps
                thunks = [
                    (lambda e=e, pas=pas, heads=heads: emit_pv(
                        e[0], e[1], pas, heads, e[2]))
                    for e in pend
                ]
                for hs in (0, 1):
                    thunks.append(
                        lambda hs=hs, pas=pas, heads=heads, ip=ip: emit_norm(
                            (pas[hs],), (heads[hs],), via_pool=(ip < 3))
                    )
                carry = thunks

            # ---- tail: last pair's PVs + norms first (they gate the whole
            # output chain) ----
            for t in carry:  # PV(7), the two norms
                t()

            if debug_dump:
                nc.sync.dma_start(dbg["d_yT"], yT.bitcast(F32))
                nc.sync.dma_start(dbg["d_qT"], qT.bitcast(F32))
                nc.sync.dma_start(dbg["d_kT"], kT.bitcast(F32))
                dvf = big.tile([128, 8, 264], F32, tag="dvf")
                nc.vector.tensor_copy(dvf, vsb)
                nc.sync.dma_start(dbg["d_v"], dvf)
                daf = big.tile([128, 8, 256], F32, tag="daf")
                nc.vector.tensor_copy(daf, a_sb)
                nc.sync.dma_start(dbg["d_asb"], daf)

            # transpose chunk-0 (shared-bank, half-copies so the first
            # projections start before the second half lands), project in
            # token-chunk pairs, re-add staged half via identity matmul,
            # merged copies, store
            tp0 = pst.tile([128, 1024], BF16, tag="ps", name="atp0")
            for i in range(4):
                emit_atr_mm(0, i, tp0)
            for i in range(4, 8):
                emit_atr_mm(0, i, tp0)
            nc.vector.tensor_copy(attnT[:, 0, 0:512], tp0[:, 0:512])
            nc.vector.tensor_copy(attnT[:, 0, 512:1024], tp0[:, 512:1024])
            for np_ in range(4):
                ops = pst.tile([128, 2, 512], F32, tag="ps", name="ops")
                for q in range(2):
                    nt = np_ * 2 + q
                    dst = ops[:, 0, q * 256:(q + 1) * 256]
                    for kc in range(2):
                        nc.tensor.matmul(
                            dst,
                            lhsT=attnT[:, kc, nt * 128:(nt + 1) * 128],
                            rhs=outwT_sb[:, kc, :],
                            start=(q == 0 and kc == 0),
                            stop=False,
                        )
                # out_b as a K=1 tap over the whole pair bank
                ob = bass.AP(
                    tensor=outb_sb.tensor, offset=outb_sb.offset,
                    ap=[list(outb_sb.ap[0]), [0, 2], [1, 256]],
                )
                nc.tensor.matmul(
                    ops[:, 0, :],
                    lhsT=ones_sb[0:1, 0:128],
                    rhs=ob,
                    start=False,
                    stop=True,
                )
                osb2 = outs_p.tile([128, 2, C], F32, tag="o", name="osb2")
                # alternate engines: DVE is idle once the last norms are done
                copy_alt(osb2.rearrange("p a b -> p (a b)"), ops[:, 0, :])
                # one batched DMA per 2 token chunks (HWDGE overhead is
                # per-descriptor-set, ~625ns each)
                oq = nc.sync if np_ % 2 else nc.scalar
                oq.dma_start(
                    out_d[np_ * 256:(np_ + 1) * 256, :].rearrange(
                        "(c p) f -> p c f", p=128),
                    osb2,
                )

    nc.compile()
    return nc


_NC = None
LAST_RESULTS = None


def _host_prep(conv_w, conv_b, qkv_w, out_w, out_b):
    import ml_dtypes

    conv_w = np.asarray(conv_w, np.float32).reshape(C, 3, 3)
    w18 = np.zeros((128, 18), np.float32)
    for ct in range(2):
        for t, (ky, kx) in enumerate(TAPS):
            d = conv_w[128 * ct: 128 * (ct + 1), ky, kx].copy()
            if (ky, kx) == (1, 1):
                d += 1.0  # residual connection folded into the center tap
            w18[:, ct * 9 + t] = d
    blobA = np.zeros((128, BAW), ml_dtypes.bfloat16)
    blobA[:, BA_ID:BA_ID + 128] = np.eye(128, dtype=ml_dtypes.bfloat16)
    blobA[:, BA_W18:BA_W18 + 18] = w18.astype(ml_dtypes.bfloat16)
    cb = np.asarray(conv_b, np.float32).reshape(2, 128).T
    blobA[:, BA_CONVB:BA_CONVB + 2] = cb.astype(ml_dtypes.bfloat16)
    blobB = np.zeros((128, BBW), ml_dtypes.bfloat16)
    owT = np.ascontiguousarray(np.asarray(out_w, np.float32).T).astype(
        ml_dtypes.bfloat16)  # [256 in, 256 outc]
    blobB[:, BB_OWT:BB_OWT + 512] = np.concatenate(
        [owT[0:128, :], owT[128:256, :]], axis=1)
    blobB[0, BB_OUTB:BB_OUTB + 256] = np.asarray(out_b, np.float32).astype(
        ml_dtypes.bfloat16)
    return {
        "qkv_wT": np.ascontiguousarray(np.asarray(qkv_w, np.float32).T),
        "blobA": blobA,
        "blobB": blobB,
    }


def _prep_x(x):
    """bf16, host-transposed to [B, C, N] for straight (transpose-free) DMA."""
    import ml_dtypes

    xt = np.swapaxes(np.asarray(x, np.float32), -1, -2)
    return np.ascontiguousarray(xt.astype(ml_dtypes.bfloat16))


def kernel(x, conv_w, conv_b, qkv_w, out_w, out_b):
    global _NC, LAST_RESULTS

    if _NC is None:
        _NC = build_nc()
    x = _prep_x(x)
    shared = _host_prep(conv_w, conv_b, qkv_w, out_w, out_b)
    in_maps = [{**shared, "x": np.ascontiguousarray(x[b])} for b in range(B)]
    trace = bool(int(os.environ.get("KERNEL_TRACE", "0")))
    try:
        res = run_bass_kernel_spmd(_NC, in_maps, core_ids=list(range(B)), trace=trace)
    except Exception:
        if not trace:
            raise
        res = run_bass_kernel_spmd(_NC, in_maps, core_ids=list(range(B)), trace=False)
    LAST_RESULTS = res
    return np.stack([res.results[b]["out"] for b in range(B)], axis=0)


# revision 84
# speedup vs baseline: 1.0651x; 1.0219x over previous
"""Trainium2 Bass kernel for nn_Attention_43190191129190.

Model (per batch element b of 8):
    y   = x + dwconv3x3(x) + conv_b          (depthwise residual positional conv)
    qkv = y @ qkv_w.T ; split into q, k, v   (8 heads, dim 32)
    out = softmax(q k^T / sqrt(32)) v
    out = out @ out_w.T + out_b
Sharding: pure data-parallel, one batch element per NeuronCore (8 cores).

Per-core design (v5 — merged 2-bank tiles, wide exp, merged evacuations):

  The ACT+DVE engines are the wall: every S element must be exp'd
  (65536 cols of [128]-partition work) and every PSUM result must be
  evacuated by ACT/DVE (DMA and GPSIMD cannot touch PSUM). v5 cuts the
  per-instruction init overhead (ACT ~185ns, DVE ~125ns busy per op) by
  merging work into the widest possible instructions:

  1. x arrives bf16, host pre-transposed; 2 DMA-xbar transposes stage
     x^T, copied into a zero-haloed [C, 34, 34] image.
  2. diag conv matrices built on device by TWO [128,9,128]
     affine_selects (one per channel tile) instead of 18 narrow ones.
  3. conv per ct: one [128,2,512] PSUM tile, j halves as two 10-matmul
     accumulation groups, ONE [128,1024] evacuation (bias via K=1 tap).
  4. q^T/k^T per feature tile: one [128,2,512] tile, 4 matmuls, ONE
     evacuation. v: two 4-token-chunk units, 8 matmuls + ONE strided
     evacuation each into [v_h|1] 33-wide head slots (ones preset).
  5. Attention, head pair per generation, 8 m-steps each:
       S^T per (head, m): one [128,2,512] f32 PSUM tile (two 512-wide
       matmuls), then ONE 1024-wide exp:
         hs0 -> ACT exact Exp -> bf16; hs1 -> DVE Schraudolph
         (tensor_scalar s*A+B -> int16 bits == bf16(exp(s*SCALE))).
       PV unchanged: per-head [128, 8x33] PSUM accumulator, stationary
       p^T chunks, moving [v_h|1]; column 32 = softmax denominators;
       one accumulation group per bank. Norm per head: reciprocal +
       one broadcast tensor_tensor -> a_sb bf16.
  6. a_sb -> attnT: 8 transposes per ct share ONE [128,1024] bf16 bank
     (single accumulation group, disjoint regions), ONE 2x-mode DVE
     copy per ct.
  7. projection: chunk-1 + out_b staged mid-kernel into partial1 (pairs
     of token chunks share a bank, one [128,512] copy); tail re-adds
     partial1 via f32r identity matmuls and stores via merged copies.

  PSUM: 3x[128,2,512] f32 rotating slots + 2x[128,264] PV accumulators.
  Pre-attention work interleaves into the pair loops one self-contained
  slice per m-step (alloc+use+evacuate within the slice).
"""

import os

import numpy as np

import concourse.bass as bass
import concourse.tile as tile
from concourse import bacc, mybir
from concourse.bass_utils import run_bass_kernel_spmd

F32 = mybir.dt.float32
F32R = mybir.dt.float32r
BF16 = mybir.dt.bfloat16
I16 = mybir.dt.int16
AF = mybir.ActivationFunctionType
ALU = mybir.AluOpType

B, N, C = 8, 1024, 256
HEADS, DH = 8, 32
SCALE = DH ** -0.5
PAD = 34  # 32x32 spatial grid with 1-px halo

# blobA (bf16): id [128, 0:128] | w18 [128, 128:146] | convb cols [128, 146:148]
BA_ID, BA_W18, BA_CONVB, BAW = 0, 128, 146, 148
# blobB (bf16): outwT [128, 0:512] | outb row0 [512:768]
BB_OWT, BB_OUTB, BBW = 0, 512, 768

TAPS = [(ky, kx) for ky in range(3) for kx in range(3)]
# chunk-1 head pairs first so the chunk-1 projection can run mid-kernel;
# the tail then only waits on the last pair's (chunk-0) normalization
PAIRS = [(5, 7), (4, 6), (1, 3), (0, 2)]

# Schraudolph fast-exp: int16 bits of bf16(exp(s*SCALE)) = s*A + B
SCHR_C = 450000.0
SCHR_A = float(SCALE * (2 ** 23) / np.log(2) / 65536.0)
SCHR_B = float((127 * 2 ** 23 - SCHR_C) / 65536.0)


def build_nc(debug_dump=False):
    nc = bacc.Bacc("TRN2", target_bir_lowering=False, debug=False, num_devices=8)

    # x arrives host-pre-transposed: [C, N] bf16, one straight DMA
    x_d = nc.dram_tensor("x", (C, N), BF16, kind="ExternalInput").ap()
    qkvwT_d = nc.dram_tensor("qkv_wT", (C, 3 * C), F32R, kind="ExternalInput").ap()
    blobA_d = nc.dram_tensor("blobA", (128, BAW), BF16, kind="ExternalInput").ap()
    blobB_d = nc.dram_tensor("blobB", (128, BBW), BF16, kind="ExternalInput").ap()
    out_d = nc.dram_tensor("out", (N, C), F32, kind="ExternalOutput").ap()
    dbg = {}
    if debug_dump:
        for name, shape in (
            ("d_yT", (128, 2, N)), ("d_qT", (128, 2, N)), ("d_kT", (128, 2, N)),
            ("d_v", (128, 8, 264)), ("d_asb", (128, 8, 256)),
        ):
            dbg[name] = nc.dram_tensor(name, shape, F32, kind="ExternalOutput").ap()

    with tile.TileContext(nc) as tc:
        with (
            tc.tile_pool(name="const", bufs=1) as const,
            tc.tile_pool(name="big", bufs=1) as big,
            tc.tile_pool(name="pT", bufs=16) as ppool,
            tc.tile_pool(name="rcp", bufs=4) as rcp_p,
            tc.tile_pool(name="outs", bufs=4) as outs_p,
            tc.tile_pool(name="pst", bufs=3, space="PSUM") as pst,
            tc.tile_pool(name="pap", bufs=2, space="PSUM") as pap,
        ):
            # ---- persistent activations (x image first: DMA critical path)
            xpadT = big.tile([128, 2, PAD * PAD], BF16, tag="xpadT")
            xpv = xpadT.bitcast(mybir.dt.uint16).rearrange(
                "p ct (h w) -> p ct h w", h=PAD
            )
            nc.vector.memset(xpv[:, :, 0, :], 0)
            nc.vector.memset(xpv[:, :, PAD - 1, :], 0)
            nc.vector.memset(xpv[:, :, :, 0], 0)
            nc.vector.memset(xpv[:, :, :, PAD - 1], 0)

            # ---- DMAs. Per-DMA cost in the serial DMA pipeline is large
            # (HWDGE 625 + DGE delay 650 + transfer + completion sem 900),
            # so x is host-pre-transposed and lands in ONE straight DMA.
            blobA_sb = const.tile([128, BAW], BF16, tag="blobA")
            nc.sync.dma_start(blobA_sb, blobA_d)
            xstg = big.tile([128, 2, N], BF16, tag="xstg")
            for ct in range(2):
                nc.sync.dma_start(xstg[:, ct, :],
                                  x_d[ct * 128:(ct + 1) * 128, :])
            id_sb = blobA_sb[:, BA_ID:BA_ID + 128]
            w18_sb = blobA_sb[:, BA_W18:BA_W18 + 18]
            convb2_sb = blobA_sb[:, BA_CONVB:BA_CONVB + 2]
            qkvwT_sb = const.tile([128, 2, 3 * C], F32R, tag="qkvwT")
            for ct in range(2):
                nc.sync.dma_start(
                    qkvwT_sb[:, ct, 0:512],
                    qkvwT_d[ct * 128:(ct + 1) * 128, 0:512],
                )
            nc.sync.dma_start(
                qkvwT_sb[:, :, 512:768],
                qkvwT_d[:, 512:768].rearrange("(kc p) f -> p kc f", p=128),
            )
            blobB_sb = const.tile([128, BBW], BF16, tag="blobB")
            nc.sync.dma_start(blobB_sb, blobB_d)
            outwT_sb = blobB_sb[:, BB_OWT:BB_OWT + 512].rearrange(
                "p (kc f) -> p kc f", kc=2)
            outb_sb = blobB_sb[0:1, BB_OUTB:BB_OUTB + 256]

            # diag conv matrices: diag[c, t, f] = w18[c, t] * id[c, f] via
            # one DVE tensor_tensor per channel tile (DVE is idle at startup
            # and this beats the Pool affine_select by ~3us of latency)
            # ---- warm-ups (after the DMA issues so they don't block the
            # ACT queue): the exp ACT-table load and a chained trickle of
            # tiny PE matmuls (keeps the PE "recently active" through the
            # DMA wait so the conv burst is not dispatched into the cost
            # model's cold p-state)
            zerob_sb = const.tile([128, 1], F32, tag="zerob")
            nc.vector.memset(zerob_sb, 0.0)
            warm_sb = const.tile([1, 1], F32, tag="warm")
            nc.scalar.activation(
                warm_sb, zerob_sb[0:1, 0:1], AF.Exp,
                bias=zerob_sb[0:1], scale=1.0,
            )
            wv = const.tile([1, 20], F32, tag="wv")
            nc.vector.memset(wv, 0.0)
            for k in range(17):
                wps = pst.tile([128, 2, 512], F32, tag="ps", name="wps")
                nc.tensor.matmul(
                    wps[0:1, 0, 0:1], lhsT=wv[0:1, k:k + 1],
                    rhs=wv[0:1, k:k + 1], start=True, stop=True,
                )
                if k + 1 < 20:
                    nc.scalar.copy(wv[0:1, k + 1:k + 2], wps[0:1, 0, 0:1])

            diag_sb = const.tile([128, 18, 128], BF16, tag="diag")

            def emit_diag(ct):
                idb = bass.AP(
                    tensor=id_sb.tensor, offset=id_sb.offset,
                    ap=[list(id_sb.ap[0]), [0, 9], [1, 128]],
                )
                w18b = bass.AP(
                    tensor=w18_sb.tensor,
                    offset=w18_sb.offset + ct * 9,
                    ap=[list(w18_sb.ap[0]), [1, 9], [0, 128]],
                )
                nc.vector.tensor_tensor(
                    out=diag_sb[:, ct * 9:(ct + 1) * 9, :],
                    in0=idb, in1=w18b, op=ALU.mult,
                )

            def emit_xpad(ct):
                nc.vector.tensor_copy(
                    xpadT[:, ct, :].rearrange("p (h w) -> p h w", h=PAD)[
                        :, 1:33, 1:33
                    ],
                    xstg[:, ct, :].rearrange("p (h w) -> p h w", h=32),
                )

            # ones row generated on device (proj-bias rhs)
            ones_sb = const.tile([1, 512], BF16, tag="ones")
            nc.gpsimd.memset(ones_sb, 1.0)
            # conv bias in f32 for the per-partition bias of the conv
            # evacuation (folds the bias add into the PSUM->SBUF copy)
            convbf = const.tile([128, 2], F32, tag="convbf")
            # DVE order matters: ct0's conv inputs complete before ct1's
            # begin, so the ct0 conv matmuls start ~2us sooner
            nc.vector.tensor_copy(convbf, convb2_sb)
            emit_diag(0)
            emit_xpad(0)
            emit_diag(1)
            emit_xpad(1)

            yT = big.tile([128, 2, N], F32R, tag="yT")
            qT = big.tile([128, 2, N], F32R, tag="qT")
            kT = big.tile([128, 2, N], F32R, tag="kT")
            # [v_h | 1] per (token-chunk, head); ones preset via memset
            vsb = big.tile([128, 8, 8 * 33], BF16, tag="v")
            nc.gpsimd.memset(vsb, 1.0)
            a_sb = big.tile([128, 8, 256], BF16, tag="a_sb")
            attnT = big.tile([128, 2, N], BF16, tag="attnT")

            # psum evacuations: GPSIMD cannot access PSUM on HW, so they
            # alternate between the ACT (scalar.copy) and DVE engines
            _cp = [0]

            def copy_alt(dst, src_ap):
                _cp[0] += 1
                if _cp[0] % 2:
                    nc.scalar.copy(dst, src_ap)
                else:
                    nc.vector.tensor_copy(dst, src_ap)

            # ---- conv: per (ct, j) half: 9 diagonal matmuls + K=1 bias tap,
            # one 512-wide evacuation (j-split so the attention wavefront can
            # start on the j0 token half while j1 is still convolving)
            def emit_conv_half(ct, j):
                cps = pst.tile([128, 512], F32, tag="ps", name=f"cacc{ct}{j}")
                view = xpadT[:, ct, :].rearrange("p (h w) -> p h w", h=PAD)
                for t, (ky, kx) in enumerate(TAPS):
                    nc.tensor.matmul(
                        cps,
                        lhsT=diag_sb[:, ct * 9 + t, :],
                        rhs=view[:, ky + 16 * j: ky + 16 * j + 16, kx: kx + 32],
                        start=(t == 0),
                        stop=(t == 8),
                    )
                # conv bias folded into the evacuation (per-partition add)
                dst = yT[:, ct, j * 512:(j + 1) * 512]
                _cp[0] += 1
                if _cp[0] % 2:
                    nc.scalar.activation(
                        dst, cps, AF.Identity,
                        bias=convbf[:, ct:ct + 1], scale=1.0)
                else:
                    cb = bass.AP(
                        tensor=convbf.tensor, offset=convbf.offset + ct,
                        ap=[list(convbf.ap[0]), [0, 512]],
                    )
                    nc.vector.tensor_tensor(
                        out=dst, in0=cps, in1=cb, op=ALU.add)

            # ---- q^T / k^T: full feature tile or single token-half ----
            def emit_qk_half(ft, j):
                dstT, dc = (qT, ft) if ft < 2 else (kT, ft - 2)
                fofs = 0 if ft < 2 else 256
                qps = pst.tile([128, 512], F32, tag="ps", name="qps")
                for kc in range(2):
                    nc.tensor.matmul(
                        qps,
                        lhsT=qkvwT_sb[:, kc, fofs + dc * 128: fofs + (dc + 1) * 128],
                        rhs=yT[:, kc, j * 512:(j + 1) * 512],
                        start=(kc == 0),
                        stop=(kc == 1),
                    )
                copy_alt(dstT[:, dc, j * 512:(j + 1) * 512], qps)

            def emit_qk(ft):
                dstT, dc = (qT, ft) if ft < 2 else (kT, ft - 2)
                fofs = 0 if ft < 2 else 256
                qps = pst.tile([128, 2, 512], F32, tag="ps", name="qps")
                for j in range(2):
                    for kc in range(2):
                        nc.tensor.matmul(
                            qps[:, j, :],
                            lhsT=qkvwT_sb[:, kc, fofs + dc * 128: fofs + (dc + 1) * 128],
                            rhs=yT[:, kc, j * 512:(j + 1) * 512],
                            start=(kc == 0),
                            stop=(kc == 1),
                        )
                copy_alt(dstT[:, dc, :], qps.rearrange("p a b -> p (a b)"))

            # ---- v: 4 token chunks per unit, 8 matmuls, one strided evac ----
            def emit_v4(u):
                vps = pst.tile([128, 2, 512], F32, tag="ps", name="vps")
                for q in range(4):
                    nt = u * 4 + q
                    dst = vps[:, q // 2, (q % 2) * 256:(q % 2) * 256 + 256]
                    for kc in range(2):
                        # one open accumulation group per bank: start on the
                        # bank's first write, stop on its last
                        nc.tensor.matmul(
                            dst,
                            lhsT=yT[:, kc, nt * 128:(nt + 1) * 128],
                            rhs=qkvwT_sb[:, kc, 512:768],
                            start=(kc == 0 and q % 2 == 0),
                            stop=(kc == 1 and q % 2 == 1),
                        )
                sv = vps.rearrange("p a (q hh c) -> p (a q) hh c", q=2, c=32)
                dv = vsb[:, u * 4:(u + 1) * 4, :].rearrange(
                    "p n (hh c) -> p n hh c", c=33)[:, :, :, 0:32]
                copy_alt(dv, sv)

            # pre-loop: exactly what pair 0 needs up front (chunk-1 q/k and
            # the first four v chunks); the rest trickles in as one light
            # half-unit extra per m-step so the S/exp PSUM rotation is never
            # starved for long
            for ct in range(2):
                for j in range(2):
                    emit_conv_half(ct, j)
            emit_qk(1)
            emit_qk(3)
            emit_v4(0)

            # ---- a_sb -> attnT: 8 transposes sharing one bank + ONE copy ----
            def emit_atr_mm(ct, nc_i, tp):
                nc.tensor.matmul(
                    tp[:, nc_i * 128:(nc_i + 1) * 128],
                    lhsT=a_sb[:, nc_i, ct * 128:(ct + 1) * 128],
                    rhs=id_sb,
                    is_transpose=True,
                    start=(nc_i == 0),
                    stop=(nc_i == 7),
                )

            # interleaved extras, one self-contained slice per m-step
            def emit_atr_ct(ct):
                # all 8 transposes share one bank-tile + ONE 2x-mode copy;
                # single slice keeps the PSUM slot hold under ~1 m-step
                tp = pst.tile([128, 1024], BF16, tag="ps", name=f"atp{ct}")
                for i in range(8):
                    emit_atr_mm(ct, i, tp)
                nc.vector.tensor_copy(attnT[:, ct, :], tp)

            def pair_extra(ip, m):
                if ip == 0:
                    if m == 1:
                        emit_v4(1)
                    elif m == 3:
                        emit_qk_half(0, 0)
                    elif m == 5:
                        emit_qk_half(0, 1)
                elif ip == 1:
                    if m == 1:
                        emit_qk_half(2, 0)
                    elif m == 3:
                        emit_qk_half(2, 1)
                elif ip == 2:
                    if m == 6:
                        emit_atr_ct(1)

            # ---- merged exp: ONE 1024-wide instruction per (head, m).
            # hs0 sits on the 1-step-slack PSUM slot: its exp gates the
            # S-issue chain, so it always runs on the faster ACT engine.
            # hs1 (2-step slack) goes to DVE except two steps per pair,
            # balancing total engine busy (~42 ACT / 22 DVE tiles).
            def emit_exp_half(eng, sv, w):
                if eng == "A":
                    p = ppool.tile([128, w], BF16, tag="pT", name="pA")
                    nc.scalar.activation(p, sv, AF.Exp, bias=zerob_sb, scale=SCALE)
                    return p
                p = ppool.tile([128, w], I16, tag="pT", name="pV")
                nc.vector.tensor_scalar(
                    out=p, in0=sv, scalar1=SCHR_A, scalar2=SCHR_B,
                    op0=ALU.mult, op1=ALU.add,
                )
                return p.bitcast(BF16)

            def emit_exp(eng, st2):
                return emit_exp_half(
                    eng, st2.rearrange("p a b -> p (a b)"), 1024)

            # ---- attention ----
            def emit_pv(m, ph, pas, heads, rng=None):
                # one accumulation group per pa bank: start only on the first
                # write (lazy 2KB region-zeroing covers the other 7
                # sub-regions), stop only on the last. rng selects a 4-chunk
                # n-range for the pair-0 wavefront half-tiles.
                base = 0 if rng is None else rng
                for nc_i in (range(8) if rng is None else range(rng, rng + 4)):
                    for hs in (0, 1):
                        nc.tensor.matmul(
                            pas[hs][:, nc_i * 33: nc_i * 33 + 33],
                            lhsT=ph[hs][:, (nc_i - base) * 128:
                                        (nc_i - base + 1) * 128],
                            rhs=vsb[:, m, 33 * heads[hs]: 33 * heads[hs] + 33],
                            start=(m == 0 and nc_i == 0),
                            stop=(m == 7 and nc_i == 7),
                        )

            def emit_norm(pas, heads, via_pool=False):
                for h, pa in zip(heads, pas):
                    pav = pa.rearrange("p (nc e) -> p nc e", e=33)
                    rcp = rcp_p.tile([128, 8], F32, tag="rcp", name="rcp")
                    if via_pool:
                        # DVE is the loop's ceiling engine: stage the PSUM
                        # accumulator to SBUF on ACT, then run reciprocal +
                        # broadcast multiply on the otherwise-idle Pool
                        psb = rcp_p.tile([128, 264], F32, tag="nrm",
                                         name="psb")
                        nc.scalar.copy(psb, pa)
                        pv = psb.rearrange("p (nc e) -> p nc e", e=33)
                        nc.vector.reciprocal(rcp, pv[:, :, 32])
                        rcp_b = bass.AP(
                            tensor=rcp.tensor, offset=rcp.offset,
                            ap=[list(rcp.ap[0]), [1, 8], [0, 32]],
                        )
                        nc.gpsimd.tensor_tensor(
                            out=a_sb[:, :, h * 32: h * 32 + 32],
                            in0=pv[:, :, 0:32],
                            in1=rcp_b,
                            op=ALU.mult,
                        )
                    else:
                        nc.vector.reciprocal(rcp, pav[:, :, 32])
                        rcp_b = bass.AP(
                            tensor=rcp.tensor, offset=rcp.offset,
                            ap=[list(rcp.ap[0]), [1, 8], [0, 32]],
                        )
                        nc.vector.tensor_tensor(
                            out=a_sb[:, :, h * 32: h * 32 + 32],
                            in0=pav[:, :, 0:32],
                            in1=rcp_b,
                            op=ALU.mult,
                        )

            def emit_s_half(h, m, j):
                a = 32 * (h % 4)
                hc = h // 4
                sth = pst.tile([128, 512], F32, tag="ps", name="sth")
                nc.tensor.matmul(
                    sth,
                    lhsT=kT[a:a + 32, hc, m * 128:(m + 1) * 128],
                    rhs=qT[a:a + 32, hc, j * 512:(j + 1) * 512],
                    start=True,
                    stop=True,
                    tile_position=(a, 0),
                )
                return sth

            def emit_s_full(h, m):
                a = 32 * (h % 4)
                hc = h // 4
                st2 = pst.tile([128, 2, 512], F32, tag="ps", name="st")
                for j in range(2):
                    nc.tensor.matmul(
                        st2[:, j, :],
                        lhsT=kT[a:a + 32, hc, m * 128:(m + 1) * 128],
                        rhs=qT[a:a + 32, hc, j * 512:(j + 1) * 512],
                        start=True,
                        stop=True,
                        tile_position=(a, 0),
                    )
                return st2

            carry = []
            for ip, (hA, hB) in enumerate(PAIRS):
                pas = (
                    pap.tile([128, 264], F32, tag="pa", name=f"paA{ip}"),
                    pap.tile([128, 264], F32, tag="pa", name=f"paB{ip}"),
                )
                heads = (hA, hB)
                pend = []
                for m in range(8):
                    ph = {
                        hs: emit_exp(
                            "A" if hs == 0 else "V", emit_s_full(h, m))
                        for hs, h in ((0, hA), (1, hB))
                    }
                    pend.append((m, ph, None))
                    # carried PVs wait until m>=2 so the previous pair's
                    # trailing exps (still draining on DVE) don't head-of-line
                    # stall the PE queue
                    if carry and m >= 2:
                        carry.pop(0)()
                    pair_extra(ip, m)
                    # the last pair drains its PVs earlier to shorten the tail
                    if len(pend) > (1 if ip == 3 else 2):
                        e = pend.pop(0)
                        emit_pv(e[0], e[1], pas, heads, e[2])
                # defer the tail PVs + normalization into the next pair's
                # m-loop so the PE never waits on the trailing exps
                thunks = [
                    (lambda e=e, pas=pas, heads=heads: emit_pv(
                        e[0], e[1], pas, heads, e[2]))
                    for e in pend
                ]
                for hs in (0, 1):
                    thunks.append(
                        lambda hs=hs, pas=pas, heads=heads, ip=ip: emit_norm(
                            (pas[hs],), (heads[hs],), via_pool=(ip < 3))
                    )
                carry = thunks

            # ---- tail: last pair's PVs + norms first (they gate the whole
            # output chain) ----
            for t in carry:  # PV(7), the two norms
                t()

            if debug_dump:
                nc.sync.dma_start(dbg["d_yT"], yT.bitcast(F32))
                nc.sync.dma_start(dbg["d_qT"], qT.bitcast(F32))
                nc.sync.dma_start(dbg["d_kT"], kT.bitcast(F32))
                dvf = big.tile([128, 8, 264], F32, tag="dvf")
                nc.vector.tensor_copy(dvf, vsb)
                nc.sync.dma_start(dbg["d_v"], dvf)
                daf = big.tile([128, 8, 256], F32, tag="daf")
                nc.vector.tensor_copy(daf, a_sb)
                nc.sync.dma_start(dbg["d_asb"], daf)

            # transpose chunk-0 (shared-bank, half-copies so the first
            # projections start before the second half lands), project in
            # token-chunk pairs, re-add staged half via identity matmul,
            # merged copies, store
            tp0 = pst.tile([128, 1024], BF16, tag="ps", name="atp0")
            for i in range(4):
                emit_atr_mm(0, i, tp0)
            for i in range(4, 8):
                emit_atr_mm(0, i, tp0)
            nc.vector.tensor_copy(attnT[:, 0, 0:512], tp0[:, 0:512])
            nc.vector.tensor_copy(attnT[:, 0, 512:1024], tp0[:, 512:1024])
            for np_ in range(4):
                ops = pst.tile([128, 2, 512], F32, tag="ps", name="ops")
                for q in range(2):
                    nt = np_ * 2 + q
                    dst = ops[:, 0, q * 256:(q + 1) * 256]
                    for kc in range(2):
                        nc.tensor.matmul(
                            dst,
                            lhsT=attnT[:, kc, nt * 128:(nt + 1) * 128],
                            rhs=outwT_sb[:, kc, :],
                            start=(q == 0 and kc == 0),
                            stop=False,
                        )
                # out_b as a K=1 tap over the whole pair bank
                ob = bass.AP(
                    tensor=outb_sb.tensor, offset=outb_sb.offset,
                    ap=[list(outb_sb.ap[0]), [0, 2], [1, 256]],
                )
                nc.tensor.matmul(
                    ops[:, 0, :],
                    lhsT=ones_sb[0:1, 0:128],
                    rhs=ob,
                    start=False,
                    stop=True,
                )
                osb2 = outs_p.tile([128, 2, C], F32, tag="o", name="osb2")
                # alternate engines: DVE is idle once the last norms are done
                copy_alt(osb2.rearrange("p a b -> p (a b)"), ops[:, 0, :])
                # one batched DMA per 2 token chunks (HWDGE overhead is
                # per-descriptor-set, ~625ns each)
                oq = nc.sync if np_ % 2 else nc.scalar
                oq.dma_start(
                    out_d[np_ * 256:(np_ + 1) * 256, :].rearrange(
                        "(c p) f -> p c f", p=128),
                    osb2,
                )

    nc.compile()
    return nc


_NC = None
LAST_RESULTS = None


def _host_prep(conv_w, conv_b, qkv_w, out_w, out_b):
    import ml_dtypes

    conv_w = np.asarray(conv_w, np.float32).reshape(C, 3, 3)
    w18 = np.zeros((128, 18), np.float32)
    for ct in range(2):
        for t, (ky, kx) in enumerate(TAPS):
            d = conv_w[128 * ct: 128 * (ct + 1), ky, kx].copy()
            if (ky, kx) == (1, 1):
                d += 1.0  # residual connection folded into the center tap
            w18[:, ct * 9 + t] = d
    blobA = np.zeros((128, BAW), ml_dtypes.bfloat16)
    blobA[:, BA_ID:BA_ID + 128] = np.eye(128, dtype=ml_dtypes.bfloat16)
    blobA[:, BA_W18:BA_W18 + 18] = w18.astype(ml_dtypes.bfloat16)
    cb = np.asarray(conv_b, np.float32).reshape(2, 128).T
    blobA[:, BA_CONVB:BA_CONVB + 2] = cb.astype(ml_dtypes.bfloat16)
    blobB = np.zeros((128, BBW), ml_dtypes.bfloat16)
    owT = np.ascontiguousarray(np.asarray(out_w, np.float32).T).astype(
        ml_dtypes.bfloat16)  # [256 in, 256 outc]
    blobB[:, BB_OWT:BB_OWT + 512] = np.concatenate(
        [owT[0:128, :], owT[128:256, :]], axis=1)
    blobB[0, BB_OUTB:BB_OUTB + 256] = np.asarray(out_b, np.float32).astype(
        ml_dtypes.bfloat16)
    return {
        "qkv_wT": np.ascontiguousarray(np.asarray(qkv_w, np.float32).T),
        "blobA": blobA,
        "blobB": blobB,
    }


def _prep_x(x):
    """bf16, host-transposed to [B, C, N] for straight (transpose-free) DMA."""
    import ml_dtypes

    xt = np.swapaxes(np.asarray(x, np.float32), -1, -2)
    return np.ascontiguousarray(xt.astype(ml_dtypes.bfloat16))


def kernel(x, conv_w, conv_b, qkv_w, out_w, out_b):
    global _NC, LAST_RESULTS

    if _NC is None:
        _NC = build_nc()
    x = _prep_x(x)
    shared = _host_prep(conv_w, conv_b, qkv_w, out_w, out_b)
    in_maps = [{**shared, "x": np.ascontiguousarray(x[b])} for b in range(B)]
    trace = bool(int(os.environ.get("KERNEL_TRACE", "0")))
    try:
        res = run_bass_kernel_spmd(_NC, in_maps, core_ids=list(range(B)), trace=trace)
    except Exception:
        if not trace:
            raise
        res = run_bass_kernel_spmd(_NC, in_maps, core_ids=list(range(B)), trace=False)
    LAST_RESULTS = res
    return np.stack([res.results[b]["out"] for b in range(B)], axis=0)
